# revision 1
# baseline (speedup 1.0000x reference)
"""Trainium2 Bass kernel for ConfigurableNoisyQuantumLayer.

Math: the circuit is a fixed sequence of single-qubit rotations, CNOTs and
noise channels acting on an 8-qubit density matrix, batched over 32 inputs x.
In the (real) Pauli-transfer-matrix picture every channel is a real 4^n x 4^n
matrix. We pull the observable Z_0 back through the 6 layers (Heisenberg
picture) -- one shared real (4^4)x(4^4)=256x256 matrix chain independent of
the batch -- then contract with per-sample product-state Pauli vectors.

Per adjoint layer (l = 5..0), with q the 256x256 pullback matrix
(rows = wires 0-3 pair-index, cols = wires 4-7):
    T  = A_l q          A_l = kron_{w=0..3} F(l,w)^T
    W  = T B_l          B_l = kron_{w=4..7} F(l,w)
    q' = sum_k E_k W D_k       (k = 0..3: rank-4 Schmidt split of the one
                                row/col-crossing CNOT pair; E_k, D_k are
                                constants that also absorb the row-local and
                                col-local CNOT+noise blocks)
F(l,w) = N1 @ blkdiag(1, Rz(t_z) Ry(t_y)) is the per-wire rotation+noise PTM.
Output: out[b] = P_r(b)^T q P_c(b) with P_r/P_c kron products of per-wire
encoding vectors N1 @ (1, sin x, 0, cos x).

Each of the 8 cores runs the identical chain and handles 4 of the 32 samples.
"""

import os
import sys

import numpy as np

sys.path.insert(0, "/opt/trn_rl_repo")

import concourse.bass as bass  # noqa: E402
import concourse.bacc as bacc  # noqa: E402
import concourse.tile as tile  # noqa: E402
from concourse import mybir  # noqa: E402

F32 = mybir.dt.float32
F32R = mybir.dt.float32r
AF = mybir.ActivationFunctionType

N_QUBITS = 8
DEPTH = 6
BATCH = 32
N_CORES = 8
B_PER = BATCH // N_CORES  # 4
G1, G2 = 0.0003, 0.0065

HALF_PI = float(np.pi / 2)

# ---------------------------------------------------------------------------
# Constant precompute (numpy, float64 -> float32)
# ---------------------------------------------------------------------------


def _consts():
    I2 = np.eye(2, dtype=complex)
    X = np.array([[0, 1], [1, 0]], dtype=complex)
    Y = np.array([[0, -1j], [1j, 0]], dtype=complex)
    Z = np.diag([1.0, -1.0]).astype(complex)
    PAULI = [I2, X, Y, Z]

    def amp_k(g):
        return [np.array([[1, 0], [0, np.sqrt(1 - g)]], complex),
                np.array([[0, np.sqrt(g)], [0, 0]], complex)]

    def phase_k(g):
        return [np.array([[1, 0], [0, np.sqrt(1 - g)]], complex),
                np.array([[0, 0], [0, np.sqrt(g)]], complex)]

    def depol_k(p):
        s0, s = np.sqrt(1 - p), np.sqrt(p / 3.0)
        return [s0 * I2, s * X, s * Y, s * Z]

    def super_1q(kraus):
        S = np.zeros((4, 4))
        for a in range(4):
            for b in range(4):
                acc = 0j
                for K in kraus:
                    acc += np.trace(PAULI[a] @ K @ PAULI[b] @ K.conj().T)
                S[a, b] = (0.5 * acc).real
        return S

    def chan(chs):
        S = np.eye(4)
        for k in chs:
            S = super_1q(k) @ S
        return S

    N1 = chan([amp_k(G1 * 0.3), phase_k(G1 * 0.2), depol_k(G1 * 0.5)])
    N2 = chan([amp_k(G2 * 0.3), phase_k(G2 * 0.2), depol_k(G2 * 0.5)])

    CNOT = np.array(
        [[1, 0, 0, 0], [0, 1, 0, 0], [0, 0, 0, 1], [0, 0, 1, 0]], complex)
    S_CNOT = np.zeros((16, 16))
    for a1 in range(4):
        for a2 in range(4):
            PA = np.kron(PAULI[a1], PAULI[a2])
            for b1 in range(4):
                for b2 in range(4):
                    PB = np.kron(PAULI[b1], PAULI[b2])
                    S_CNOT[4 * a1 + a2, 4 * b1 + b2] = (
                        0.25 * np.trace(PA @ CNOT @ PB @ CNOT.conj().T)).real
    C2 = np.kron(N2, N2) @ S_CNOT
    C2T = C2.T

    def lift(M, pos):  # on 4 base-4 digits, digit 0 most significant
        return np.kron(np.kron(np.eye(4 ** pos), M), np.eye(4 ** (2 - pos)))

    G_c = lift(C2T, 0) @ lift(C2T, 1) @ lift(C2T, 2)
    G_r = lift(C2T, 0) @ lift(C2T, 1) @ lift(C2T, 2)  # same structure

    C4 = C2T.reshape(4, 4, 4, 4)
    R = C4.transpose(0, 2, 1, 3).reshape(16, 16)
    U, s, Vt = np.linalg.svd(R)
    rank = int((s > 1e-12).sum())
    assert rank == 4, rank
    Ds = np.zeros((4, 256, 256))
    EsT = np.zeros((4, 256, 256))
    for k in range(4):
        alpha = (np.sqrt(s[k]) * U[:, k]).reshape(4, 4)
        beta = (np.sqrt(s[k]) * Vt[k, :]).reshape(4, 4)
        E_k = G_r @ np.kron(np.eye(64), alpha)
        D_k = (np.kron(beta, np.eye(64)) @ G_c).T
        Ds[k] = D_k
        EsT[k] = E_k.T

    # selectors
    S4 = np.zeros((4, 16))
    S4t = np.zeros((4, 16))
    for p in range(16):
        S4[p >> 2, p] = 1.0
        S4t[p & 3, p] = 1.0
    S16h = np.zeros((2, 16, 128))
    S16t = np.zeros((16, 128))
    for c in range(2):
        for p in range(128):
            S16h[c, (128 * c + p) >> 4, p] = 1.0
    for p in range(128):
        S16t[p & 15, p] = 1.0

    f = np.float32
    return dict(
        N1T=N1.T.astype(f), Ds=Ds.astype(f), EsT=EsT.astype(f),
        S4=S4.astype(f), S4t=S4t.astype(f),
        S16h=S16h.astype(f), S16t=S16t.astype(f),
    )

# ---------------------------------------------------------------------------
# Bass kernel builder
# ---------------------------------------------------------------------------


def build_nc(mm_fast=True):
    """One NeuronCore program: inputs xp [4,8], wt [6,8,2] -> out [4,1]."""
    C = _consts()
    mmdt = F32R if mm_fast else F32

    nc = bacc.Bacc("TRN2", target_bir_lowering=False, debug=False,
                   num_devices=N_CORES)
    xp = nc.declare_dram_parameter("xp", [B_PER, N_QUBITS], F32, isOutput=False)
    wt = nc.declare_dram_parameter("wt", [DEPTH, N_QUBITS, 2], F32, isOutput=False)
    out_d = nc.declare_dram_parameter("out", [B_PER, 1], F32, isOutput=True)

    dN1c = nc.inline_tensor(C["N1T"].reshape(1, 16).copy(), "cN1c")
    dS4 = nc.inline_tensor(C["S4"], "cS4")
    dS4t = nc.inline_tensor(C["S4t"], "cS4t")
    dS16h = nc.inline_tensor(C["S16h"], "cS16h")
    dS16t = nc.inline_tensor(C["S16t"], "cS16t")
    qinit = np.zeros((256, 256), np.float32)
    qinit[192, 0] = 1.0
    dQ0 = nc.inline_tensor(qinit, "cQ0")
    # D stacked along columns: Dst[r, 256*k + j] = D_k[r, j]
    Dst = np.ascontiguousarray(C["Ds"].transpose(1, 0, 2).reshape(256, 1024))
    dDst = nc.inline_tensor(Dst, "cDst")
    dEsT = nc.inline_tensor(C["EsT"], "cEsT")

    with tile.TileContext(nc) as tc:
        with (
            tc.tile_pool(name="cpool", bufs=1) as cpool,
            tc.tile_pool(name="abpool", bufs=1) as abpool,
            tc.tile_pool(name="wpool", bufs=2) as wpool,
            tc.tile_pool(name="qpool", bufs=2) as qpool,
            tc.tile_pool(name="ppmm", bufs=4, space="PSUM") as ppmm,
            tc.tile_pool(name="ppsm", bufs=2, space="PSUM") as ppsm,
        ):
            def cdma(dram_ap, shape, tag, dt_=F32):
                t = cpool.tile(shape, dt_, tag=tag, name=tag)
                if dt_ is F32:
                    nc.sync.dma_start(t[:], dram_ap)
                else:
                    nc.sync.dma_start(t[:], dram_ap.bitcast(dt_))
                return t

            tN1c = cdma(dN1c[:, :], [1, 16], "n1c")
            tS4 = cdma(dS4[:, :], [4, 16], "s4")
            tS4t = cdma(dS4t[:, :], [4, 16], "s4t")
            tS16h = [cdma(dS16h[c], [16, 128], f"s16h{c}") for c in range(2)]
            tS16t = cdma(dS16t[:, :], [16, 128], "s16t")
            tDst = [cdma(dDst[128 * c:128 * (c + 1), :], [128, 1024],
                         f"dst{c}", mmdt) for c in range(2)]
            tEsT = [[cdma(dEsT[k, 128 * c:128 * (c + 1), :], [128, 256],
                          f"es{k}{c}", mmdt) for c in range(2)] for k in range(4)]

            tones = cpool.tile([128, 1], F32, tag="ones", name="ones")
            nc.vector.memset(tones[:], 1.0)
            tpi2 = cpool.tile([1, 1], F32, tag="pi2", name="pi2")
            nc.vector.memset(tpi2[:], HALF_PI)
            tone_row = cpool.tile([1, 48], F32, tag="tone_row", name="tone_row")
            nc.vector.memset(tone_row[:], 1.0)

            # ---------------- angles -> F_all [4, 192] ----------------
            th = cpool.tile([1, 96], F32, tag="th", name="th")
            nc.sync.dma_start(
                th[:].rearrange("p (t j) -> p t j", t=2),
                wt[:].rearrange("l w t -> () t (l w)"))
            sn = cpool.tile([1, 96], F32, tag="sn", name="sn")
            cs = cpool.tile([1, 96], F32, tag="cs", name="cs")
            nc.scalar.activation(sn[:], th[:], AF.Sin)
            nc.scalar.activation(cs[:], th[:], AF.Sin, bias=tpi2[:])
            sy, szr = sn[0:1, 0:48], sn[0:1, 48:96]
            cy, czr = cs[0:1, 0:48], cs[0:1, 48:96]
            pcc = cpool.tile([1, 48], F32, tag="pcc", name="pcc")  # cz*cy
            pcs = cpool.tile([1, 48], F32, tag="pcs", name="pcs")  # cz*sy
            psc = cpool.tile([1, 48], F32, tag="psc", name="psc")  # sz*cy
            pss = cpool.tile([1, 48], F32, tag="pss", name="pss")  # sz*sy
            nc.vector.tensor_mul(pcc[:], czr, cy)
            nc.vector.tensor_mul(pcs[:], czr, sy)
            nc.vector.tensor_mul(psc[:], szr, cy)
            nc.vector.tensor_mul(pss[:], szr, sy)

            # Rotblk rows as [1,192] vectors (r0..r3), then
            # F_all = sum_r N1[:, r] (x) row_r  via K=1 accumulating matmuls.
            rv = []
            for r in range(4):
                t = cpool.tile([1, 192], F32, tag=f"rv{r}", name=f"rv{r}")
                nc.vector.memset(t[:], 0.0)
                rv.append(t)
            rvv = [t[:].rearrange("p (j n) -> p j n", n=4) for t in rv]

            def c3(a):
                return a.rearrange("p j -> p j ()")

            nc.vector.tensor_copy(rvv[0][:, :, 0:1], c3(tone_row[0:1, :]))
            nc.vector.tensor_copy(rvv[1][:, :, 1:2], c3(pcc[:]))
            nc.scalar.mul(rvv[1][:, :, 2:3], c3(szr), -1.0)
            nc.vector.tensor_copy(rvv[1][:, :, 3:4], c3(pcs[:]))
            nc.vector.tensor_copy(rvv[2][:, :, 1:2], c3(psc[:]))
            nc.vector.tensor_copy(rvv[2][:, :, 2:3], c3(czr))
            nc.vector.tensor_copy(rvv[2][:, :, 3:4], c3(pss[:]))
            nc.scalar.mul(rvv[3][:, :, 1:2], c3(sy), -1.0)
            nc.vector.tensor_copy(rvv[3][:, :, 3:4], c3(cy))

            ps_f = ppsm.tile([4, 192], F32, tag="sm", name="ps_f")
            for r in range(4):
                nc.tensor.matmul(ps_f[:], tN1c[0:1, 4 * r:4 * (r + 1)], rv[r][:],
                                 start=(r == 0), stop=(r == 3))
            fall = cpool.tile([4, 192], F32, tag="fall", name="fall")
            nc.vector.tensor_copy(fall[:], ps_f[:])

            # ------------- batched selector expansions -------------
            # t1a[p, 4j+n] = F_j[p>>2, n]; t2a[p, 4j+n] = F_j[p&3, n]
            ps1 = ppsm.tile([16, 192], F32, tag="sm", name="ps1")
            nc.tensor.matmul(ps1[:], tS4[:], fall[:], start=True, stop=True)
            t1a = cpool.tile([16, 192], F32, tag="t1a", name="t1a")
            nc.scalar.copy(t1a[:], ps1[:])
            ps2 = ppsm.tile([16, 192], F32, tag="sm", name="ps2")
            nc.tensor.matmul(ps2[:], tS4t[:], fall[:], start=True, stop=True)
            t2a = cpool.tile([16, 192], F32, tag="t2a", name="t2a")
            nc.scalar.copy(t2a[:], ps2[:])

            # pair-kron tiles for all layers: fpa[pos][p, 16l + 4a+b]
            fpa = []
            for pos in range(4):
                fp = abpool.tile([16, 96], F32, tag=f"fpa{pos}", name=f"fpa{pos}")
                for l in range(DEPTH):
                    o = 32 * l + 8 * pos
                    nc.vector.tensor_mul(
                        fp[:, 16 * l:16 * (l + 1)].rearrange(
                            "p (a b) -> p a b", a=4),
                        t1a[:, o:o + 4].unsqueeze(2).broadcast_to([16, 4, 4]),
                        t2a[:, o + 4:o + 8].unsqueeze(1).broadcast_to([16, 4, 4]),
                    )
                fpa.append(fp)

            # quad selector expansions, batched over layers: [128, 96]
            def sel_expand(sel, fp_all, tag):
                ps = ppsm.tile([128, 96], F32, tag="sm", name=f"ps{tag}")
                nc.tensor.matmul(ps[:], sel[:], fp_all[:], start=True, stop=True)
                t = cpool.tile([128, 96], F32, tag=tag, name=tag)
                nc.scalar.copy(t[:], ps[:])
                return t

            zA = [sel_expand(tS16h[c], fpa[0], f"zA{c}") for c in range(2)]
            yA = sel_expand(tS16t, fpa[1], "yA")
            zB = [sel_expand(tS16h[c], fpa[2], f"zB{c}") for c in range(2)]
            yB = sel_expand(tS16t, fpa[3], "yB")

            # A/B kron tiles per layer (DVE broadcast muls, all-SBUF inputs)
            At = {}
            Bt = {}
            for l in range(DEPTH):
                sl = slice(16 * l, 16 * (l + 1))
                At[l] = []
                Bt[l] = []
                for c in range(2):
                    ab = abpool.tile([128, 256], mmdt, tag=f"A{l}_{c}",
                                     name=f"A{l}_{c}")
                    nc.vector.tensor_mul(
                        ab[:].rearrange("p (a b) -> p a b", a=16),
                        zA[c][:, sl].unsqueeze(2).broadcast_to([128, 16, 16]),
                        yA[:, sl].unsqueeze(1).broadcast_to([128, 16, 16]),
                    )
                    At[l].append(ab)
                    bb = abpool.tile([128, 256], mmdt, tag=f"B{l}_{c}",
                                     name=f"B{l}_{c}")
                    nc.vector.tensor_mul(
                        bb[:].rearrange("p (a b) -> p a b", a=16),
                        zB[c][:, sl].unsqueeze(2).broadcast_to([128, 16, 16]),
                        yB[:, sl].unsqueeze(1).broadcast_to([128, 16, 16]),
                    )
                    Bt[l].append(bb)

            # ---------------- encoding vectors ----------------
            sx = cpool.tile([1, 32], F32, tag="sx", name="sx")
            nc.sync.dma_start(sx[:], xp[:].rearrange("b w -> () (b w)"))
            xsin = cpool.tile([1, 32], F32, tag="xsin", name="xsin")
            xcos = cpool.tile([1, 32], F32, tag="xcos", name="xcos")
            nc.scalar.activation(xsin[:], sx[:], AF.Sin)
            nc.scalar.activation(xcos[:], sx[:], AF.Sin, bias=tpi2[:])
            ones32 = cpool.tile([1, 32], F32, tag="ones32", name="ones32")
            nc.vector.memset(ones32[:], 1.0)
            ps_e = ppsm.tile([4, 32], F32, tag="sm", name="ps_e")
            for i, (r, src_row) in enumerate([(0, ones32), (1, xsin), (3, xcos)]):
                nc.tensor.matmul(ps_e[:], tN1c[0:1, 4 * r:4 * (r + 1)], src_row[:],
                                 start=(i == 0), stop=(i == 2))
            aenc = cpool.tile([4, 32], F32, tag="aenc", name="aenc")
            nc.vector.tensor_copy(aenc[:], ps_e[:])

            pse1 = ppsm.tile([16, 32], F32, tag="sm", name="pse1")
            nc.tensor.matmul(pse1[:], tS4[:], aenc[:], start=True, stop=True)
            s1e = cpool.tile([16, 32], F32, tag="s1e", name="s1e")
            nc.scalar.copy(s1e[:], pse1[:])
            pse2 = ppsm.tile([16, 32], F32, tag="sm", name="pse2")
            nc.tensor.matmul(pse2[:], tS4t[:], aenc[:], start=True, stop=True)
            s2e = cpool.tile([16, 32], F32, tag="s2e", name="s2e")
            nc.scalar.copy(s2e[:], pse2[:])

            def wcol(t, w):
                return t[:].rearrange("p (b w) -> p b w", w=8)[:, :, w]

            # ahi = [a01 | a45], alo = [a23 | a67]  (cols = 4 samples each)
            ahi = cpool.tile([16, 8], F32, tag="ahi", name="ahi")
            alo = cpool.tile([16, 8], F32, tag="alo", name="alo")
            nc.vector.tensor_mul(ahi[:, 0:4], wcol(s1e, 0), wcol(s2e, 1))
            nc.vector.tensor_mul(ahi[:, 4:8], wcol(s1e, 4), wcol(s2e, 5))
            nc.vector.tensor_mul(alo[:, 0:4], wcol(s1e, 2), wcol(s2e, 3))
            nc.vector.tensor_mul(alo[:, 4:8], wcol(s1e, 6), wcol(s2e, 7))

            psy = ppsm.tile([128, 8], F32, tag="sm", name="psy")
            nc.tensor.matmul(psy[:], tS16t[:], alo[:], start=True, stop=True)
            yq = cpool.tile([128, 8], F32, tag="yq", name="yq")
            nc.scalar.copy(yq[:], psy[:])
            Pr = []
            Pc = []
            for c in range(2):
                psz = ppsm.tile([128, 8], F32, tag="sm", name="psz")
                nc.tensor.matmul(psz[:], tS16h[c][:], ahi[:], start=True, stop=True)
                pr = cpool.tile([128, B_PER], F32, tag=f"pr{c}", name=f"pr{c}")
                nc.vector.tensor_mul(pr[:], psz[:, 0:4], yq[:, 0:4])
                pc = cpool.tile([128, B_PER], F32, tag=f"pc{c}", name=f"pc{c}")
                nc.vector.tensor_mul(pc[:], psz[:, 4:8], yq[:, 4:8])
                Pr.append(pr)
                Pc.append(pc)

            # ---------------- q init ----------------
            q_sb = []
            for c in range(2):
                t = qpool.tile([128, 256], mmdt, tag=f"q{c}", name=f"q{c}")
                src_ap = dQ0[128 * c:128 * (c + 1), :]
                if mmdt is not F32:
                    src_ap = src_ap.bitcast(mmdt)
                nc.sync.dma_start(t[:], src_ap)
                q_sb.append(t)

            # ---------------- the 6-layer chain ----------------
            def mm(dst_psum, lhsT, rhs, start, stop):
                nc.tensor.matmul(dst_psum, lhsT.bitcast(mmdt), rhs.bitcast(mmdt),
                                 start=start, stop=stop)

            copy_flip = [0]

            def copy_out(dst, src):
                if copy_flip[0] % 2 == 0:
                    nc.vector.tensor_copy(dst, src)
                else:
                    nc.scalar.copy(dst, src)
                copy_flip[0] += 1

            for s in range(DEPTH):
                l = DEPTH - 1 - s
                # Tp = q^T @ A   [C, R']
                tp_sb = []
                for m in range(2):
                    ps = ppmm.tile([128, 256], F32, tag="mm", name="ps_tp")
                    for c in range(2):
                        mm(ps[:], q_sb[c][:, 128 * m:128 * (m + 1)], At[l][c][:],
                           start=(c == 0), stop=(c == 1))
                    t = wpool.tile([128, 256], mmdt, tag=f"tp{m}", name=f"tp{m}")
                    copy_out(t[:], ps[:])
                    tp_sb.append(t)
                # Wp = B^T @ Tp  [C', R']
                wp_sb = []
                for m in range(2):
                    ps = ppmm.tile([128, 256], F32, tag="mm", name="ps_wp")
                    for c in range(2):
                        mm(ps[:], Bt[l][c][:, 128 * m:128 * (m + 1)], tp_sb[c][:],
                           start=(c == 0), stop=(c == 1))
                    t = wpool.tile([128, 256], mmdt, tag=f"wp{m}", name=f"wp{m}")
                    copy_out(t[:], ps[:])
                    wp_sb.append(t)
                # U = W @ [D_0|D_1|D_2|D_3]   [R', (k,j)] as [128, 1024] tiles
                uall = []
                for m in range(2):
                    u = wpool.tile([128, 1024], mmdt, tag=f"u{m}", name=f"u{m}")
                    for nh in range(2):
                        ps = ppmm.tile([128, 512], F32, tag="mm", name="ps_u")
                        for c in range(2):
                            mm(ps[:], wp_sb[c][:, 128 * m:128 * (m + 1)],
                               tDst[c][:, 512 * nh:512 * (nh + 1)],
                               start=(c == 0), stop=(c == 1))
                        copy_out(u[:, 512 * nh:512 * (nh + 1)], ps[:])
                    uall.append(u)
                # q' = sum_k E_k U_k
                q_new = []
                for m in range(2):
                    ps = ppmm.tile([128, 256], F32, tag="mm", name="ps_q")
                    first = True
                    for k in range(4):
                        for c in range(2):
                            mm(ps[:], tEsT[k][c][:, 128 * m:128 * (m + 1)],
                               uall[c][:, 256 * k:256 * (k + 1)],
                               start=first, stop=(k == 3 and c == 1))
                            first = False
                    t = qpool.tile([128, 256], mmdt, tag=f"q{m}", name=f"q{m}")
                    copy_out(t[:], ps[:])
                    q_new.append(t)
                q_sb = q_new

            # ---------------- final contraction ----------------
            h_sb = []
            for m in range(2):
                ps = ppsm.tile([128, B_PER], F32, tag="sm", name="ps_g")
                for c in range(2):
                    nc.tensor.matmul(
                        ps[:], q_sb[c][:, 128 * m:128 * (m + 1)].bitcast(F32),
                        Pr[c][:], start=(c == 0), stop=(c == 1))
                h = cpool.tile([128, B_PER], F32, tag=f"h{m}", name=f"h{m}")
                nc.vector.tensor_mul(h[:], ps[:], Pc[m][:])
                h_sb.append(h)
            ps_o = ppsm.tile([B_PER, 1], F32, tag="sm", name="ps_o")
            for m in range(2):
                nc.tensor.matmul(ps_o[:], h_sb[m][:], tones[:],
                                 start=(m == 0), stop=(m == 1))
            out_sb = cpool.tile([B_PER, 1], F32, tag="osb", name="osb")
            nc.vector.tensor_copy(out_sb[:], ps_o[:])
            nc.sync.dma_start(out_d[:, :], out_sb[:])

    nc.compile()
    return nc


# ---------------------------------------------------------------------------
# Host entry point
# ---------------------------------------------------------------------------

_NC = None


def _get_nc():
    global _NC
    if _NC is None:
        _NC = build_nc(mm_fast=os.environ.get("QK_MM_FP32") != "1")
    return _NC


def _maybe_enable_ldw_opt():
    if os.environ.get("QK_LDW_OPT") != "1":
        return
    from concourse.compiler_utils import get_compiler_flags, set_compiler_flags

    flags = [f.replace("--enable-ldw-opt=false", "--enable-ldw-opt=true")
             for f in get_compiler_flags()]
    set_compiler_flags(flags)


def kernel(x: np.ndarray, weights: np.ndarray) -> np.ndarray:
    from concourse.bass_utils import run_bass_kernel_spmd

    _maybe_enable_ldw_opt()

    nc = _get_nc()
    x = np.ascontiguousarray(x, dtype=np.float32)
    weights = np.ascontiguousarray(weights, dtype=np.float32)
    in_maps = [
        {"xp": x[i * B_PER:(i + 1) * B_PER], "wt": weights}
        for i in range(N_CORES)
    ]
    res = run_bass_kernel_spmd(nc, in_maps, list(range(N_CORES)))
    out = np.concatenate([res.results[i]["out"] for i in range(N_CORES)], axis=0)
    return out.astype(np.float32)



# revision 12
# speedup vs baseline: 1.1197x; 1.1197x over previous
"""Trainium2 Bass kernel for ConfigurableNoisyQuantumLayer.

Math: the circuit is a fixed sequence of single-qubit rotations, CNOTs and
noise channels acting on an 8-qubit density matrix, batched over 32 inputs x.
In the (real) Pauli-transfer-matrix picture every channel is a real 4^n x 4^n
matrix. We pull the observable Z_0 back through the 6 layers (Heisenberg
picture) -- one shared real (4^4)x(4^4)=256x256 matrix chain independent of
the batch -- then contract with per-sample product-state Pauli vectors.

Per adjoint layer (l = 5..0), with q the 256x256 pullback matrix
(rows = wires 0-3 pair-index, cols = wires 4-7):
    T  = A_l q          A_l = kron_{w=0..3} F(l,w)^T
    W  = T B_l          B_l = kron_{w=4..7} F(l,w)
    q' = sum_k E_k W D_k       (k = 0..3: rank-4 Schmidt split of the one
                                row/col-crossing CNOT pair; E_k, D_k are
                                constants that also absorb the row-local and
                                col-local CNOT+noise blocks)
F(l,w) = N1 @ blkdiag(1, Rz(t_z) Ry(t_y)) is the per-wire rotation+noise PTM.
Output: out[b] = P_r(b)^T q P_c(b) with P_r/P_c kron products of per-wire
encoding vectors N1 @ (1, sin x, 0, cos x).

Each of the 8 cores runs the identical chain and handles 4 of the 32 samples.
"""

import os
import sys

import numpy as np

sys.path.insert(0, "/opt/trn_rl_repo")

import concourse.bass as bass  # noqa: E402
import concourse.bacc as bacc  # noqa: E402
import concourse.tile as tile  # noqa: E402
from concourse import mybir  # noqa: E402

F32 = mybir.dt.float32
F32R = mybir.dt.float32r
F16 = mybir.dt.float16
AF = mybir.ActivationFunctionType

N_QUBITS = 8
DEPTH = 6
BATCH = 32
N_CORES = 8
B_PER = BATCH // N_CORES  # 4
G1, G2 = 0.0003, 0.0065

HALF_PI = float(np.pi / 2)

# ---------------------------------------------------------------------------
# Constant precompute (numpy, float64 -> float32)
# ---------------------------------------------------------------------------


def _consts():
    I2 = np.eye(2, dtype=complex)
    X = np.array([[0, 1], [1, 0]], dtype=complex)
    Y = np.array([[0, -1j], [1j, 0]], dtype=complex)
    Z = np.diag([1.0, -1.0]).astype(complex)
    PAULI = [I2, X, Y, Z]

    def amp_k(g):
        return [np.array([[1, 0], [0, np.sqrt(1 - g)]], complex),
                np.array([[0, np.sqrt(g)], [0, 0]], complex)]

    def phase_k(g):
        return [np.array([[1, 0], [0, np.sqrt(1 - g)]], complex),
                np.array([[0, 0], [0, np.sqrt(g)]], complex)]

    def depol_k(p):
        s0, s = np.sqrt(1 - p), np.sqrt(p / 3.0)
        return [s0 * I2, s * X, s * Y, s * Z]

    def super_1q(kraus):
        S = np.zeros((4, 4))
        for a in range(4):
            for b in range(4):
                acc = 0j
                for K in kraus:
                    acc += np.trace(PAULI[a] @ K @ PAULI[b] @ K.conj().T)
                S[a, b] = (0.5 * acc).real
        return S

    def chan(chs):
        S = np.eye(4)
        for k in chs:
            S = super_1q(k) @ S
        return S

    N1 = chan([amp_k(G1 * 0.3), phase_k(G1 * 0.2), depol_k(G1 * 0.5)])
    N2 = chan([amp_k(G2 * 0.3), phase_k(G2 * 0.2), depol_k(G2 * 0.5)])

    CNOT = np.array(
        [[1, 0, 0, 0], [0, 1, 0, 0], [0, 0, 0, 1], [0, 0, 1, 0]], complex)
    S_CNOT = np.zeros((16, 16))
    for a1 in range(4):
        for a2 in range(4):
            PA = np.kron(PAULI[a1], PAULI[a2])
            for b1 in range(4):
                for b2 in range(4):
                    PB = np.kron(PAULI[b1], PAULI[b2])
                    S_CNOT[4 * a1 + a2, 4 * b1 + b2] = (
                        0.25 * np.trace(PA @ CNOT @ PB @ CNOT.conj().T)).real
    C2 = np.kron(N2, N2) @ S_CNOT
    C2T = C2.T

    def lift(M, pos):  # on 4 base-4 digits, digit 0 most significant
        return np.kron(np.kron(np.eye(4 ** pos), M), np.eye(4 ** (2 - pos)))

    G_c = lift(C2T, 0) @ lift(C2T, 1) @ lift(C2T, 2)
    G_r = lift(C2T, 0) @ lift(C2T, 1) @ lift(C2T, 2)  # same structure

    C4 = C2T.reshape(4, 4, 4, 4)
    R = C4.transpose(0, 2, 1, 3).reshape(16, 16)
    U, s, Vt = np.linalg.svd(R)
    rank = int((s > 1e-12).sum())
    assert rank == 4, rank
    Ds = np.zeros((4, 256, 256))
    EsT = np.zeros((4, 256, 256))
    for k in range(4):
        alpha = (np.sqrt(s[k]) * U[:, k]).reshape(4, 4)
        beta = (np.sqrt(s[k]) * Vt[k, :]).reshape(4, 4)
        E_k = G_r @ np.kron(np.eye(64), alpha)
        D_k = (np.kron(beta, np.eye(64)) @ G_c).T
        Ds[k] = D_k
        EsT[k] = E_k.T

    # selectors
    S4 = np.zeros((4, 16))
    S4t = np.zeros((4, 16))
    for p in range(16):
        S4[p >> 2, p] = 1.0
        S4t[p & 3, p] = 1.0
    S16h = np.zeros((2, 16, 128))
    S16t = np.zeros((16, 128))
    for c in range(2):
        for p in range(128):
            S16h[c, (128 * c + p) >> 4, p] = 1.0
    for p in range(128):
        S16t[p & 15, p] = 1.0

    f = np.float32
    return dict(
        N1T=N1.T.astype(f), Ds=Ds.astype(f), EsT=EsT.astype(f),
        S4=S4.astype(f), S4t=S4t.astype(f),
        S16h=S16h.astype(f), S16t=S16t.astype(f),
    )

# ---------------------------------------------------------------------------
# Bass kernel builder
# ---------------------------------------------------------------------------


def build_nc(mm_fast=True):
    """One NeuronCore program: inputs xp [4,8], wt [6,8,2] -> out [4,1]."""
    C = _consts()
    mmdt = F16
    h = np.float16

    nc = bacc.Bacc("TRN2", target_bir_lowering=False, debug=False,
                   num_devices=N_CORES)
    xp = nc.declare_dram_parameter("xp", [B_PER, N_QUBITS], F32, isOutput=False)
    wt = nc.declare_dram_parameter("wt", [DEPTH, N_QUBITS, 2], F32, isOutput=False)
    out_d = nc.declare_dram_parameter("out", [B_PER, 1], F32, isOutput=True)

    dN1c = nc.inline_tensor(C["N1T"].reshape(1, 16).astype(h), "cN1c")
    dS4 = nc.inline_tensor(C["S4"].astype(h), "cS4")
    dS4t = nc.inline_tensor(C["S4t"].astype(h), "cS4t")
    dS16h = nc.inline_tensor(C["S16h"].astype(h), "cS16h")
    dS16t = nc.inline_tensor(C["S16t"].astype(h), "cS16t")
    # D stacked along columns: Dst[r, 256*k + j] = D_k[r, j]
    Dst = np.ascontiguousarray(C["Ds"].transpose(1, 0, 2).reshape(256, 1024))
    dDst = nc.inline_tensor(Dst.astype(h), "cDst")
    dEsT = nc.inline_tensor(C["EsT"].astype(h), "cEsT")

    with tile.TileContext(nc) as tc:
        with (
            tc.tile_pool(name="cpool", bufs=1) as cpool,
            tc.tile_pool(name="abpool", bufs=1) as abpool,
            tc.tile_pool(name="wpool", bufs=2) as wpool,
            tc.tile_pool(name="qpool", bufs=2) as qpool,
            tc.tile_pool(name="ppmm", bufs=4, space="PSUM") as ppmm,
            tc.tile_pool(name="ppsm", bufs=2, space="PSUM") as ppsm,
        ):
            def cdma(dram_ap, shape, tag, dt_=F16):
                t = cpool.tile(shape, dt_, tag=tag, name=tag)
                nc.sync.dma_start(t[:], dram_ap)
                return t

            tDst = [cdma(dDst[128 * c:128 * (c + 1), :], [128, 1024],
                         f"dst{c}") for c in range(2)]
            tEsT = [[cdma(dEsT[k, 128 * c:128 * (c + 1), :], [128, 256],
                          f"es{k}{c}") for c in range(2)] for k in range(4)]
            tN1c = cdma(dN1c[:, :], [1, 16], "n1c")
            tS4 = cdma(dS4[:, :], [4, 16], "s4")
            tS4t = cdma(dS4t[:, :], [4, 16], "s4t")
            tS16h = [cdma(dS16h[c], [16, 128], f"s16h{c}") for c in range(2)]
            tS16t = cdma(dS16t[:, :], [16, 128], "s16t")

            tones = cpool.tile([128, 1], F16, tag="ones", name="ones")
            nc.vector.memset(tones[:], 1.0)
            tpi2 = cpool.tile([1, 1], F32, tag="pi2", name="pi2")
            nc.vector.memset(tpi2[:], HALF_PI)
            tone_row = cpool.tile([1, 48], F32, tag="tone_row", name="tone_row")
            nc.vector.memset(tone_row[:], 1.0)

            # ---------------- angles -> F_all [4, 192] ----------------
            th = cpool.tile([1, 96], F32, tag="th", name="th")
            nc.sync.dma_start(
                th[:].rearrange("p (t j) -> p t j", t=2),
                wt[:].rearrange("l w t -> () t (l w)"))
            sn = cpool.tile([1, 96], F32, tag="sn", name="sn")
            cs = cpool.tile([1, 96], F32, tag="cs", name="cs")
            nc.scalar.activation(sn[:], th[:], AF.Sin)
            nc.scalar.activation(cs[:], th[:], AF.Sin, bias=tpi2[:])
            sy, szr = sn[0:1, 0:48], sn[0:1, 48:96]
            cy, czr = cs[0:1, 0:48], cs[0:1, 48:96]
            pcc = cpool.tile([1, 48], F32, tag="pcc", name="pcc")  # cz*cy
            pcs = cpool.tile([1, 48], F32, tag="pcs", name="pcs")  # cz*sy
            psc = cpool.tile([1, 48], F32, tag="psc", name="psc")  # sz*cy
            pss = cpool.tile([1, 48], F32, tag="pss", name="pss")  # sz*sy
            nc.vector.tensor_mul(pcc[:], czr, cy)
            nc.vector.tensor_mul(pcs[:], czr, sy)
            nc.vector.tensor_mul(psc[:], szr, cy)
            nc.vector.tensor_mul(pss[:], szr, sy)

            # Rotblk rows as [1,192] vectors (r0..r3), then
            # F_all = sum_r N1[:, r] (x) row_r  via K=1 accumulating matmuls.
            rv = []
            for r in range(4):
                t = cpool.tile([1, 192], F16, tag=f"rv{r}", name=f"rv{r}")
                nc.vector.memset(t[:], 0.0)
                rv.append(t)
            rvv = [t[:].rearrange("p (j n) -> p j n", n=4) for t in rv]

            def c3(a):
                return a.rearrange("p j -> p j ()")

            nc.vector.tensor_copy(rvv[0][:, :, 0:1], c3(tone_row[0:1, :]))
            nc.vector.tensor_copy(rvv[1][:, :, 1:2], c3(pcc[:]))
            nc.scalar.mul(rvv[1][:, :, 2:3], c3(szr), -1.0)
            nc.vector.tensor_copy(rvv[1][:, :, 3:4], c3(pcs[:]))
            nc.vector.tensor_copy(rvv[2][:, :, 1:2], c3(psc[:]))
            nc.vector.tensor_copy(rvv[2][:, :, 2:3], c3(czr))
            nc.vector.tensor_copy(rvv[2][:, :, 3:4], c3(pss[:]))
            nc.scalar.mul(rvv[3][:, :, 1:2], c3(sy), -1.0)
            nc.vector.tensor_copy(rvv[3][:, :, 3:4], c3(cy))

            ps_f = ppsm.tile([4, 192], F32, tag="sm", name="ps_f")
            for r in range(4):
                nc.tensor.matmul(ps_f[:], tN1c[0:1, 4 * r:4 * (r + 1)], rv[r][:],
                                 start=(r == 0), stop=(r == 3))
            fall = cpool.tile([4, 192], F16, tag="fall", name="fall")
            nc.vector.tensor_copy(fall[:], ps_f[:])

            # ------------- batched selector expansions -------------
            # t1a[p, 4j+n] = F_j[p>>2, n]; t2a[p, 4j+n] = F_j[p&3, n]
            ps1 = ppsm.tile([16, 192], F32, tag="sm", name="ps1")
            nc.tensor.matmul(ps1[:], tS4[:], fall[:], start=True, stop=True)
            t1a = cpool.tile([16, 192], F16, tag="t1a", name="t1a")
            nc.scalar.copy(t1a[:], ps1[:])
            ps2 = ppsm.tile([16, 192], F32, tag="sm", name="ps2")
            nc.tensor.matmul(ps2[:], tS4t[:], fall[:], start=True, stop=True)
            t2a = cpool.tile([16, 192], F16, tag="t2a", name="t2a")
            nc.scalar.copy(t2a[:], ps2[:])

            # pair-kron tiles for all layers: fpa[pos][p, 16l + 4a+b]
            fpa = []
            for pos in range(4):
                fp = abpool.tile([16, 96], F16, tag=f"fpa{pos}", name=f"fpa{pos}")
                for l in range(DEPTH):
                    o = 32 * l + 8 * pos
                    nc.vector.tensor_mul(
                        fp[:, 16 * l:16 * (l + 1)].rearrange(
                            "p (a b) -> p a b", a=4),
                        t1a[:, o:o + 4].unsqueeze(2).broadcast_to([16, 4, 4]),
                        t2a[:, o + 4:o + 8].unsqueeze(1).broadcast_to([16, 4, 4]),
                    )
                fpa.append(fp)

            # quad selector expansions, batched over layers: [128, 96]
            def sel_expand(sel, fp_all, tag):
                ps = ppsm.tile([128, 96], F32, tag="sm", name=f"ps{tag}")
                nc.tensor.matmul(ps[:], sel[:], fp_all[:], start=True, stop=True)
                t = cpool.tile([128, 96], F16, tag=tag, name=tag)
                nc.scalar.copy(t[:], ps[:])
                return t

            zA = [sel_expand(tS16h[c], fpa[0], f"zA{c}") for c in range(2)]
            yA = sel_expand(tS16t, fpa[1], "yA")
            zB = [sel_expand(tS16h[c], fpa[2], f"zB{c}") for c in range(2)]
            yB = sel_expand(tS16t, fpa[3], "yB")

            # A/B kron tiles per layer (DVE broadcast muls, all-SBUF inputs)
            At = {}
            Bt = {}
            for l in range(DEPTH):
                sl = slice(16 * l, 16 * (l + 1))
                At[l] = []
                Bt[l] = []
                for c in range(2):
                    ab = abpool.tile([128, 256], mmdt, tag=f"A{l}_{c}",
                                     name=f"A{l}_{c}")
                    nc.vector.tensor_mul(
                        ab[:].rearrange("p (a b) -> p a b", a=16),
                        zA[c][:, sl].unsqueeze(2).broadcast_to([128, 16, 16]),
                        yA[:, sl].unsqueeze(1).broadcast_to([128, 16, 16]),
                    )
                    At[l].append(ab)
                    bb = abpool.tile([128, 256], mmdt, tag=f"B{l}_{c}",
                                     name=f"B{l}_{c}")
                    nc.vector.tensor_mul(
                        bb[:].rearrange("p (a b) -> p a b", a=16),
                        zB[c][:, sl].unsqueeze(2).broadcast_to([128, 16, 16]),
                        yB[:, sl].unsqueeze(1).broadcast_to([128, 16, 16]),
                    )
                    Bt[l].append(bb)

            # ---------------- encoding vectors ----------------
            sx = cpool.tile([1, 32], F32, tag="sx", name="sx")
            nc.sync.dma_start(sx[:], xp[:].rearrange("b w -> () (b w)"))
            xsin = cpool.tile([1, 32], F16, tag="xsin", name="xsin")
            xcos = cpool.tile([1, 32], F16, tag="xcos", name="xcos")
            nc.scalar.activation(xsin[:], sx[:], AF.Sin)
            nc.scalar.activation(xcos[:], sx[:], AF.Sin, bias=tpi2[:])
            ones32 = cpool.tile([1, 32], F16, tag="ones32", name="ones32")
            nc.vector.memset(ones32[:], 1.0)
            ps_e = ppsm.tile([4, 32], F32, tag="sm", name="ps_e")
            for i, (r, src_row) in enumerate([(0, ones32), (1, xsin), (3, xcos)]):
                nc.tensor.matmul(ps_e[:], tN1c[0:1, 4 * r:4 * (r + 1)], src_row[:],
                                 start=(i == 0), stop=(i == 2))
            aenc = cpool.tile([4, 32], F16, tag="aenc", name="aenc")
            nc.vector.tensor_copy(aenc[:], ps_e[:])

            pse1 = ppsm.tile([16, 32], F32, tag="sm", name="pse1")
            nc.tensor.matmul(pse1[:], tS4[:], aenc[:], start=True, stop=True)
            s1e = cpool.tile([16, 32], F16, tag="s1e", name="s1e")
            nc.scalar.copy(s1e[:], pse1[:])
            pse2 = ppsm.tile([16, 32], F32, tag="sm", name="pse2")
            nc.tensor.matmul(pse2[:], tS4t[:], aenc[:], start=True, stop=True)
            s2e = cpool.tile([16, 32], F16, tag="s2e", name="s2e")
            nc.scalar.copy(s2e[:], pse2[:])

            def wcol(t, w):
                return t[:].rearrange("p (b w) -> p b w", w=8)[:, :, w]

            # ahi = [a01 | a45], alo = [a23 | a67]  (cols = 4 samples each)
            ahi = cpool.tile([16, 8], F16, tag="ahi", name="ahi")
            alo = cpool.tile([16, 8], F16, tag="alo", name="alo")
            nc.vector.tensor_mul(ahi[:, 0:4], wcol(s1e, 0), wcol(s2e, 1))
            nc.vector.tensor_mul(ahi[:, 4:8], wcol(s1e, 4), wcol(s2e, 5))
            nc.vector.tensor_mul(alo[:, 0:4], wcol(s1e, 2), wcol(s2e, 3))
            nc.vector.tensor_mul(alo[:, 4:8], wcol(s1e, 6), wcol(s2e, 7))

            psy = ppsm.tile([128, 8], F32, tag="sm", name="psy")
            nc.tensor.matmul(psy[:], tS16t[:], alo[:], start=True, stop=True)
            yq = cpool.tile([128, 8], F16, tag="yq", name="yq")
            nc.scalar.copy(yq[:], psy[:])
            Pr = []
            Pc = []
            for c in range(2):
                psz = ppsm.tile([128, 8], F32, tag="sm", name="psz")
                nc.tensor.matmul(psz[:], tS16h[c][:], ahi[:], start=True, stop=True)
                pr = cpool.tile([128, B_PER], F16, tag=f"pr{c}", name=f"pr{c}")
                nc.vector.tensor_mul(pr[:], psz[:, 0:4], yq[:, 0:4])
                pc = cpool.tile([128, B_PER], F16, tag=f"pc{c}", name=f"pc{c}")
                nc.vector.tensor_mul(pc[:], psz[:, 4:8], yq[:, 4:8])
                Pr.append(pr)
                Pc.append(pc)

            # ---------------- q init (rank-1: single 1.0 at [192, 0]) --------
            q_sb = []
            for c in range(2):
                t = qpool.tile([128, 256], mmdt, tag=f"q{c}", name=f"q{c}")
                nc.vector.memset(t[:], 0.0)
                q_sb.append(t)
            nc.vector.memset(q_sb[1][64:65, 0:1], 1.0)

            # ---------------- the 6-layer chain ----------------
            def mm(dst_psum, lhsT, rhs, start, stop):
                nc.tensor.matmul(dst_psum, lhsT, rhs, start=start, stop=stop)

            copy_flip = [0]

            def copy_out(dst, src):
                if copy_flip[0] % 2 == 0:
                    nc.vector.tensor_copy(dst, src)
                else:
                    nc.scalar.copy(dst, src)
                copy_flip[0] += 1

            for s in range(DEPTH):
                l = DEPTH - 1 - s
                # Tp = q^T @ A   [C, R']
                tp_sb = []
                for m in range(2):
                    ps = ppmm.tile([128, 256], F32, tag="mm", name="ps_tp")
                    for c in range(2):
                        mm(ps[:], q_sb[c][:, 128 * m:128 * (m + 1)], At[l][c][:],
                           start=(c == 0), stop=(c == 1))
                    t = wpool.tile([128, 256], mmdt, tag=f"tp{m}", name=f"tp{m}")
                    copy_out(t[:], ps[:])
                    tp_sb.append(t)
                # Wp = B^T @ Tp  [C', R']
                wp_sb = []
                for m in range(2):
                    ps = ppmm.tile([128, 256], F32, tag="mm", name="ps_wp")
                    for c in range(2):
                        mm(ps[:], Bt[l][c][:, 128 * m:128 * (m + 1)], tp_sb[c][:],
                           start=(c == 0), stop=(c == 1))
                    t = wpool.tile([128, 256], mmdt, tag=f"wp{m}", name=f"wp{m}")
                    copy_out(t[:], ps[:])
                    wp_sb.append(t)
                # U = W @ [D_0|D_1|D_2|D_3]   [R', (k,j)] as [128, 1024] tiles
                uall = []
                for m in range(2):
                    u = wpool.tile([128, 1024], mmdt, tag=f"u{m}", name=f"u{m}")
                    for nh in range(2):
                        ps = ppmm.tile([128, 512], F32, tag="mm", name="ps_u")
                        for c in range(2):
                            mm(ps[:], wp_sb[c][:, 128 * m:128 * (m + 1)],
                               tDst[c][:, 512 * nh:512 * (nh + 1)],
                               start=(c == 0), stop=(c == 1))
                        copy_out(u[:, 512 * nh:512 * (nh + 1)], ps[:])
                    uall.append(u)
                # q' = sum_k E_k U_k
                q_new = []
                for m in range(2):
                    ps = ppmm.tile([128, 256], F32, tag="mm", name="ps_q")
                    first = True
                    for k in range(4):
                        for c in range(2):
                            mm(ps[:], tEsT[k][c][:, 128 * m:128 * (m + 1)],
                               uall[c][:, 256 * k:256 * (k + 1)],
                               start=first, stop=(k == 3 and c == 1))
                            first = False
                    t = qpool.tile([128, 256], mmdt, tag=f"q{m}", name=f"q{m}")
                    copy_out(t[:], ps[:])
                    q_new.append(t)
                q_sb = q_new

            # ---------------- final contraction ----------------
            h_sb = []
            for m in range(2):
                ps = ppsm.tile([128, B_PER], F32, tag="sm", name="ps_g")
                for c in range(2):
                    nc.tensor.matmul(
                        ps[:], q_sb[c][:, 128 * m:128 * (m + 1)],
                        Pr[c][:], start=(c == 0), stop=(c == 1))
                h = cpool.tile([128, B_PER], F16, tag=f"h{m}", name=f"h{m}")
                nc.vector.tensor_mul(h[:], ps[:], Pc[m][:])
                h_sb.append(h)
            ps_o = ppsm.tile([B_PER, 1], F32, tag="sm", name="ps_o")
            for m in range(2):
                nc.tensor.matmul(ps_o[:], h_sb[m][:], tones[:],
                                 start=(m == 0), stop=(m == 1))
            out_sb = cpool.tile([B_PER, 1], F32, tag="osb", name="osb")
            nc.vector.tensor_copy(out_sb[:], ps_o[:])
            nc.sync.dma_start(out_d[:, :], out_sb[:])

    nc.compile()
    return nc


# ---------------------------------------------------------------------------
# Host entry point
# ---------------------------------------------------------------------------

_NC = None


def _get_nc():
    global _NC
    if _NC is None:
        _NC = build_nc(mm_fast=os.environ.get("QK_MM_FP32") != "1")
    return _NC


def _maybe_enable_ldw_opt():
    if os.environ.get("QK_LDW_OPT") != "1":
        return
    from concourse.compiler_utils import get_compiler_flags, set_compiler_flags

    flags = [f.replace("--enable-ldw-opt=false", "--enable-ldw-opt=true")
             for f in get_compiler_flags()]
    set_compiler_flags(flags)


def kernel(x: np.ndarray, weights: np.ndarray) -> np.ndarray:
    from concourse.bass_utils import run_bass_kernel_spmd

    _maybe_enable_ldw_opt()

    nc = _get_nc()
    x = np.ascontiguousarray(x, dtype=np.float32)
    weights = np.ascontiguousarray(weights, dtype=np.float32)
    in_maps = [
        {"xp": x[i * B_PER:(i + 1) * B_PER], "wt": weights}
        for i in range(N_CORES)
    ]
    res = run_bass_kernel_spmd(nc, in_maps, list(range(N_CORES)))
    out = np.concatenate([res.results[i]["out"] for i in range(N_CORES)], axis=0)
    return out.astype(np.float32)



# revision 17
# speedup vs baseline: 1.3545x; 1.2096x over previous
"""Trainium2 Bass kernel for ConfigurableNoisyQuantumLayer.

Math: the circuit is a fixed sequence of single-qubit rotations, CNOTs and
noise channels acting on an 8-qubit density matrix, batched over 32 inputs x.
In the (real) Pauli-transfer-matrix picture every channel is a real 4^n x 4^n
matrix. We pull the observable Z_0 back through the 6 layers (Heisenberg
picture) -- one shared real (4^4)x(4^4)=256x256 matrix chain independent of
the batch -- then contract with per-sample product-state Pauli vectors.

Per adjoint layer (l = 5..0), with q the 256x256 pullback matrix
(rows = wires 0-3 pair-index, cols = wires 4-7):
    T  = A_l q          A_l = kron_{w=0..3} F(l,w)^T
    W  = T B_l          B_l = kron_{w=4..7} F(l,w)
    q' = sum_k E_k W D_k       (k = 0..3: rank-4 Schmidt split of the one
                                row/col-crossing CNOT pair; E_k, D_k are
                                constants that also absorb the row-local and
                                col-local CNOT+noise blocks)
F(l,w) = N1 @ blkdiag(1, Rz(t_z) Ry(t_y)) is the per-wire rotation+noise PTM.
Output: out[b] = P_r(b)^T q P_c(b) with P_r/P_c kron products of per-wire
encoding vectors N1 @ (1, sin x, 0, cos x).

All matmul operands are fp16 (PE runs 1 cycle/row vs 2 for fp32; FWL weight
loads enabled). Constants ship as two blobbed fp16 inline tensors (2 DMAs).
A_l/B_l kron tiles for layer l-1 are built (DVE) while the PE crunches layer
l, so the chain starts as soon as layer 5's tiles exist.

Each of the 8 cores runs the identical chain and handles 4 of the 32 samples.
"""

import os
import sys

import numpy as np

sys.path.insert(0, "/opt/trn_rl_repo")

import concourse.bass as bass  # noqa: E402
import concourse.bacc as bacc  # noqa: E402
import concourse.tile as tile  # noqa: E402
from concourse import mybir  # noqa: E402

F32 = mybir.dt.float32
F16 = mybir.dt.float16
AF = mybir.ActivationFunctionType

N_QUBITS = 8
DEPTH = 6
BATCH = 32
N_CORES = 8
B_PER = BATCH // N_CORES  # 4
G1, G2 = 0.0003, 0.0065

HALF_PI = float(np.pi / 2)

# ---------------------------------------------------------------------------
# Constant precompute (numpy, float64 -> float16)
# ---------------------------------------------------------------------------


def _consts():
    I2 = np.eye(2, dtype=complex)
    X = np.array([[0, 1], [1, 0]], dtype=complex)
    Y = np.array([[0, -1j], [1j, 0]], dtype=complex)
    Z = np.diag([1.0, -1.0]).astype(complex)
    PAULI = [I2, X, Y, Z]

    def amp_k(g):
        return [np.array([[1, 0], [0, np.sqrt(1 - g)]], complex),
                np.array([[0, np.sqrt(g)], [0, 0]], complex)]

    def phase_k(g):
        return [np.array([[1, 0], [0, np.sqrt(1 - g)]], complex),
                np.array([[0, 0], [0, np.sqrt(g)]], complex)]

    def depol_k(p):
        s0, s = np.sqrt(1 - p), np.sqrt(p / 3.0)
        return [s0 * I2, s * X, s * Y, s * Z]

    def super_1q(kraus):
        S = np.zeros((4, 4))
        for a in range(4):
            for b in range(4):
                acc = 0j
                for K in kraus:
                    acc += np.trace(PAULI[a] @ K @ PAULI[b] @ K.conj().T)
                S[a, b] = (0.5 * acc).real
        return S

    def chan(chs):
        S = np.eye(4)
        for k in chs:
            S = super_1q(k) @ S
        return S

    N1 = chan([amp_k(G1 * 0.3), phase_k(G1 * 0.2), depol_k(G1 * 0.5)])
    N2 = chan([amp_k(G2 * 0.3), phase_k(G2 * 0.2), depol_k(G2 * 0.5)])

    CNOT = np.array(
        [[1, 0, 0, 0], [0, 1, 0, 0], [0, 0, 0, 1], [0, 0, 1, 0]], complex)
    S_CNOT = np.zeros((16, 16))
    for a1 in range(4):
        for a2 in range(4):
            PA = np.kron(PAULI[a1], PAULI[a2])
            for b1 in range(4):
                for b2 in range(4):
                    PB = np.kron(PAULI[b1], PAULI[b2])
                    S_CNOT[4 * a1 + a2, 4 * b1 + b2] = (
                        0.25 * np.trace(PA @ CNOT @ PB @ CNOT.conj().T)).real
    C2 = np.kron(N2, N2) @ S_CNOT
    C2T = C2.T

    def lift(M, pos):  # on 4 base-4 digits, digit 0 most significant
        return np.kron(np.kron(np.eye(4 ** pos), M), np.eye(4 ** (2 - pos)))

    G_c = lift(C2T, 0) @ lift(C2T, 1) @ lift(C2T, 2)
    G_r = lift(C2T, 0) @ lift(C2T, 1) @ lift(C2T, 2)  # same structure

    C4 = C2T.reshape(4, 4, 4, 4)
    R = C4.transpose(0, 2, 1, 3).reshape(16, 16)
    U, s, Vt = np.linalg.svd(R)
    rank = int((s > 1e-12).sum())
    assert rank == 4, rank
    Ds = np.zeros((4, 256, 256))
    EsT = np.zeros((4, 256, 256))
    for k in range(4):
        alpha = (np.sqrt(s[k]) * U[:, k]).reshape(4, 4)
        beta = (np.sqrt(s[k]) * Vt[k, :]).reshape(4, 4)
        E_k = G_r @ np.kron(np.eye(64), alpha)
        D_k = (np.kron(beta, np.eye(64)) @ G_c).T
        Ds[k] = D_k
        EsT[k] = E_k.T

    # selectors with N1 fused, padded to K=128 with data rows at partition 32*r
    # (Activation/engine APs must start at 32-aligned partitions)
    M1f = np.zeros((128, 16))
    M2f = np.zeros((128, 16))
    for p in range(16):
        for r in range(4):
            M1f[32 * r, p] = N1[p >> 2, r]
            M2f[32 * r, p] = N1[p & 3, r]
    # encoding: rows (1, sin, cos) -> N1 cols (0, 1, 3)
    M1e = np.zeros((128, 16))
    M2e = np.zeros((128, 16))
    for p in range(16):
        for j, r in enumerate([0, 1, 3]):
            M1e[32 * j, p] = N1[p >> 2, r]
            M2e[32 * j, p] = N1[p & 3, r]
    S16h = np.zeros((2, 16, 128))
    S16t = np.zeros((16, 128))
    for c in range(2):
        for p in range(128):
            S16h[c, (128 * c + p) >> 4, p] = 1.0
    for p in range(128):
        S16t[p & 15, p] = 1.0

    f = np.float16
    # big blob [128, 4160]: Dst tiles, EsT tiles, fused selectors
    Dst = np.ascontiguousarray(Ds.transpose(1, 0, 2).reshape(256, 1024))
    big = np.concatenate(
        [Dst[0:128], Dst[128:256]]
        + [EsT[k, 128 * c:128 * (c + 1), :] for k in range(4) for c in range(2)]
        + [M1f, M2f, M1e, M2e],
        axis=1)
    assert big.shape == (128, 4160)
    # selector blob [16, 384]
    sel = np.zeros((16, 384))
    sel[:, 0:128] = S16h[0]
    sel[:, 128:256] = S16h[1]
    sel[:, 256:384] = S16t
    return dict(big=big.astype(f), sel=sel.astype(f))

# ---------------------------------------------------------------------------
# Bass kernel builder
# ---------------------------------------------------------------------------


def build_nc():
    """One NeuronCore program: inputs xp [4,8], wt [6,8,2] -> out [4,1]."""
    C = _consts()

    nc = bacc.Bacc("TRN2", target_bir_lowering=False, debug=False,
                   num_devices=N_CORES)
    xp = nc.declare_dram_parameter("xp", [B_PER, N_QUBITS], F32, isOutput=False)
    wt = nc.declare_dram_parameter("wt", [DEPTH, N_QUBITS, 2], F32, isOutput=False)
    out_d = nc.declare_dram_parameter("out", [B_PER, 1], F32, isOutput=True)

    dBig = nc.inline_tensor(C["big"], "cBig")
    dSel = nc.inline_tensor(C["sel"], "cSel")

    with tile.TileContext(nc) as tc:
        with (
            tc.tile_pool(name="cpool", bufs=1) as cpool,
            tc.tile_pool(name="abpool", bufs=1) as abpool,
            tc.tile_pool(name="wpool", bufs=2) as wpool,
            tc.tile_pool(name="qpool", bufs=2) as qpool,
            tc.tile_pool(name="ppmm", bufs=6, space="PSUM") as ppmm,
            tc.tile_pool(name="ppsm", bufs=2, space="PSUM") as ppsm,
        ):
            # ---- input DMAs first (short; unblock the trig path asap) ----
            th = cpool.tile([1, 96], F32, tag="th", name="th")
            nc.sync.dma_start(
                th[:].rearrange("p (t j) -> p t j", t=2),
                wt[:].rearrange("l w t -> () t (l w)"))
            sx = cpool.tile([1, 32], F32, tag="sx", name="sx")
            nc.sync.dma_start(sx[:], xp[:].rearrange("b w -> () (b w)"))

            # ---- constant blobs (2 DMAs) ----
            selt = cpool.tile([16, 384], F16, tag="sel", name="sel")
            nc.sync.dma_start(selt[:], dSel[:, :])
            bigt = cpool.tile([128, 4160], F16, tag="big", name="big")
            nc.sync.dma_start(bigt[:], dBig[:, :])

            tDst = [bigt[:, 1024 * c:1024 * (c + 1)] for c in range(2)]
            tEsT = [[bigt[:, 2048 + 256 * (2 * k + c):2048 + 256 * (2 * k + c + 1)]
                     for c in range(2)] for k in range(4)]
            tS16h = [selt[0:16, 128 * c:128 * (c + 1)] for c in range(2)]
            tS16t = selt[0:16, 256:384]
            tM1f = bigt[:, 4096:4112]
            tM2f = bigt[:, 4112:4128]
            tM1e = bigt[:, 4128:4144]
            tM2e = bigt[:, 4144:4160]

            tones = cpool.tile([128, 1], F16, tag="ones", name="ones")
            nc.vector.memset(tones[:], 1.0)
            tpi2 = cpool.tile([1, 1], F32, tag="pi2", name="pi2")
            nc.vector.memset(tpi2[:], HALF_PI)

            # ---------------- angles -> rotation-block rows ----------------
            sn = cpool.tile([1, 96], F32, tag="sn", name="sn")
            cs = cpool.tile([1, 96], F32, tag="cs", name="cs")
            nc.scalar.activation(sn[:], th[:], AF.Sin)
            nc.scalar.activation(cs[:], th[:], AF.Sin, bias=tpi2[:])
            sy, szr = sn[0:1, 0:48], sn[0:1, 48:96]
            cy, czr = cs[0:1, 0:48], cs[0:1, 48:96]
            pcc = cpool.tile([1, 48], F16, tag="pcc", name="pcc")  # cz*cy
            pcs = cpool.tile([1, 48], F16, tag="pcs", name="pcs")  # cz*sy
            psc = cpool.tile([1, 48], F16, tag="psc", name="psc")  # sz*cy
            pss = cpool.tile([1, 48], F16, tag="pss", name="pss")  # sz*sy
            nc.vector.tensor_mul(pcc[:], czr, cy)
            nc.vector.tensor_mul(pcs[:], czr, sy)
            nc.vector.tensor_mul(psc[:], szr, cy)
            nc.vector.tensor_mul(pss[:], szr, sy)

            # RVall partitions 32*r hold the rotation-block row r for all (l,w)
            RVall = cpool.tile([128, 192], F16, tag="rvall", name="RVall")
            nc.vector.memset(RVall[:], 0.0)
            rvv = [RVall[32 * r:32 * r + 1, :].rearrange("p (j n) -> p j n", n=4)
                   for r in range(4)]

            def c3(a):
                return a.rearrange("p j -> p j ()")

            nc.vector.memset(rvv[0][:, :, 0:1], 1.0)
            nc.vector.tensor_copy(rvv[1][:, :, 1:2], c3(pcc[:]))
            nc.scalar.mul(rvv[1][:, :, 2:3], c3(szr), -1.0)
            nc.vector.tensor_copy(rvv[1][:, :, 3:4], c3(pcs[:]))
            nc.vector.tensor_copy(rvv[2][:, :, 1:2], c3(psc[:]))
            nc.vector.tensor_copy(rvv[2][:, :, 2:3], c3(czr))
            nc.vector.tensor_copy(rvv[2][:, :, 3:4], c3(pss[:]))
            nc.scalar.mul(rvv[3][:, :, 1:2], c3(sy), -1.0)
            nc.vector.tensor_copy(rvv[3][:, :, 3:4], c3(cy))

            # t1a[p, 4j+n] = F_j[p>>2, n]; t2a[p, 4j+n] = F_j[p&3, n]
            ps1 = ppsm.tile([16, 192], F32, tag="sm", name="ps1")
            nc.tensor.matmul(ps1[:], tM1f, RVall[:], start=True, stop=True)
            t1a = cpool.tile([16, 192], F16, tag="t1a", name="t1a")
            nc.scalar.copy(t1a[:], ps1[:])
            ps2 = ppsm.tile([16, 192], F32, tag="sm", name="ps2")
            nc.tensor.matmul(ps2[:], tM2f, RVall[:], start=True, stop=True)
            t2a = cpool.tile([16, 192], F16, tag="t2a", name="t2a")
            nc.scalar.copy(t2a[:], ps2[:])

            # pair-kron tiles for all layers: fpa[pos][p, 16l + 4a+b]
            fpa = []
            for pos in range(4):
                fp = abpool.tile([16, 96], F16, tag=f"fpa{pos}", name=f"fpa{pos}")
                for l in range(DEPTH):
                    o = 32 * l + 8 * pos
                    nc.vector.tensor_mul(
                        fp[:, 16 * l:16 * (l + 1)].rearrange(
                            "p (a b) -> p a b", a=4),
                        t1a[:, o:o + 4].unsqueeze(2).broadcast_to([16, 4, 4]),
                        t2a[:, o + 4:o + 8].unsqueeze(1).broadcast_to([16, 4, 4]),
                    )
                fpa.append(fp)

            # quad selector expansions, batched over layers: [128, 96]
            def sel_expand(sel_ap, fp_all, tag):
                ps = ppsm.tile([128, 96], F32, tag="sm", name=f"ps{tag}")
                nc.tensor.matmul(ps[:], sel_ap, fp_all[:], start=True, stop=True)
                t = cpool.tile([128, 96], F16, tag=tag, name=tag)
                nc.scalar.copy(t[:], ps[:])
                return t

            zA = [sel_expand(tS16h[c], fpa[0], f"zA{c}") for c in range(2)]
            yA = sel_expand(tS16t, fpa[1], "yA")
            zB = [sel_expand(tS16h[c], fpa[2], f"zB{c}") for c in range(2)]
            yB = sel_expand(tS16t, fpa[3], "yB")

            # A/B kron tiles for one layer (4 DVE broadcast muls)
            At = {}
            Bt = {}

            def build_ab(l):
                sl = slice(16 * l, 16 * (l + 1))
                At[l] = []
                Bt[l] = []
                for c in range(2):
                    ab = abpool.tile([128, 256], F16, tag=f"A{l}_{c}",
                                     name=f"A{l}_{c}")
                    nc.vector.tensor_mul(
                        ab[:].rearrange("p (a b) -> p a b", a=16),
                        zA[c][:, sl].unsqueeze(2).broadcast_to([128, 16, 16]),
                        yA[:, sl].unsqueeze(1).broadcast_to([128, 16, 16]),
                    )
                    At[l].append(ab)
                for c in range(2):
                    bb = abpool.tile([128, 256], F16, tag=f"B{l}_{c}",
                                     name=f"B{l}_{c}")
                    nc.vector.tensor_mul(
                        bb[:].rearrange("p (a b) -> p a b", a=16),
                        zB[c][:, sl].unsqueeze(2).broadcast_to([128, 16, 16]),
                        yB[:, sl].unsqueeze(1).broadcast_to([128, 16, 16]),
                    )
                    Bt[l].append(bb)

            build_ab(DEPTH - 1)

            # ---------------- encoding vectors ----------------
            enc3 = cpool.tile([128, 32], F16, tag="enc3", name="enc3")
            nc.vector.memset(enc3[:], 0.0)
            nc.vector.memset(enc3[0:1, :], 1.0)
            nc.scalar.activation(enc3[32:33, :], sx[:], AF.Sin)
            nc.scalar.activation(enc3[64:65, :], sx[:], AF.Sin, bias=tpi2[:])

            pse1 = ppsm.tile([16, 32], F32, tag="sm", name="pse1")
            nc.tensor.matmul(pse1[:], tM1e, enc3[:], start=True, stop=True)
            s1e = cpool.tile([16, 32], F16, tag="s1e", name="s1e")
            nc.scalar.copy(s1e[:], pse1[:])
            pse2 = ppsm.tile([16, 32], F32, tag="sm", name="pse2")
            nc.tensor.matmul(pse2[:], tM2e, enc3[:], start=True, stop=True)
            s2e = cpool.tile([16, 32], F16, tag="s2e", name="s2e")
            nc.scalar.copy(s2e[:], pse2[:])

            def wcol(t, w):
                return t[:].rearrange("p (b w) -> p b w", w=8)[:, :, w]

            # ahi = [a01 | a45], alo = [a23 | a67]  (cols = 4 samples each)
            ahi = cpool.tile([16, 8], F16, tag="ahi", name="ahi")
            alo = cpool.tile([16, 8], F16, tag="alo", name="alo")
            nc.vector.tensor_mul(ahi[:, 0:4], wcol(s1e, 0), wcol(s2e, 1))
            nc.vector.tensor_mul(ahi[:, 4:8], wcol(s1e, 4), wcol(s2e, 5))
            nc.vector.tensor_mul(alo[:, 0:4], wcol(s1e, 2), wcol(s2e, 3))
            nc.vector.tensor_mul(alo[:, 4:8], wcol(s1e, 6), wcol(s2e, 7))

            psy = ppsm.tile([128, 8], F32, tag="sm", name="psy")
            nc.tensor.matmul(psy[:], tS16t, alo[:], start=True, stop=True)
            yq = cpool.tile([128, 8], F16, tag="yq", name="yq")
            nc.scalar.copy(yq[:], psy[:])
            Pr = []
            Pc = []
            for c in range(2):
                psz = ppsm.tile([128, 8], F32, tag="sm", name="psz")
                nc.tensor.matmul(psz[:], tS16h[c], ahi[:], start=True, stop=True)
                pr = cpool.tile([128, B_PER], F16, tag=f"pr{c}", name=f"pr{c}")
                nc.vector.tensor_mul(pr[:], psz[:, 0:4], yq[:, 0:4])
                pc = cpool.tile([128, B_PER], F16, tag=f"pc{c}", name=f"pc{c}")
                nc.vector.tensor_mul(pc[:], psz[:, 4:8], yq[:, 4:8])
                Pr.append(pr)
                Pc.append(pc)

            # ---------------- q init (rank-1: single 1.0 at [192, 0]) --------
            q_sb = []
            for c in range(2):
                t = qpool.tile([128, 256], F16, tag=f"q{c}", name=f"q{c}")
                nc.vector.memset(t[:], 0.0)
                q_sb.append(t)
            nc.vector.memset(q_sb[1][64:65, 0:1], 1.0)

            # ---------------- the 6-layer chain ----------------
            def mm(dst_psum, lhsT, rhs, start, stop):
                nc.tensor.matmul(dst_psum, lhsT, rhs, start=start, stop=stop)

            copy_flip = [0]

            def copy_out(dst, src):
                if copy_flip[0] % 2 == 0:
                    nc.vector.tensor_copy(dst, src)
                else:
                    nc.scalar.copy(dst, src)
                copy_flip[0] += 1

            for s in range(DEPTH):
                l = DEPTH - 1 - s
                # Tp = q^T @ A   [C, R']  (c-outer so c=0 MMs only need q[0])
                ps_tp = [ppmm.tile([128, 256], F32, tag="mm", name=f"ps_tp{m}")
                         for m in range(2)]
                for c in range(2):
                    for m in range(2):
                        mm(ps_tp[m][:], q_sb[c][:, 128 * m:128 * (m + 1)],
                           At[l][c][:], start=(c == 0), stop=(c == 1))
                tp_sb = []
                for m in range(2):
                    t = wpool.tile([128, 256], F16, tag=f"tp{m}", name=f"tp{m}")
                    copy_out(t[:], ps_tp[m][:])
                    tp_sb.append(t)
                # Wp = B^T @ Tp  [C', R']
                ps_wp = [ppmm.tile([128, 256], F32, tag="mm", name=f"ps_wp{m}")
                         for m in range(2)]
                for c in range(2):
                    for m in range(2):
                        mm(ps_wp[m][:], Bt[l][c][:, 128 * m:128 * (m + 1)],
                           tp_sb[c][:], start=(c == 0), stop=(c == 1))
                wp_sb = []
                for m in range(2):
                    t = wpool.tile([128, 256], F16, tag=f"wp{m}", name=f"wp{m}")
                    copy_out(t[:], ps_wp[m][:])
                    wp_sb.append(t)
                # U = W @ [D_0|D_1|D_2|D_3]   [R', (k,j)] as [128, 1024] tiles
                ps_u = [[ppmm.tile([128, 512], F32, tag="mm",
                                   name=f"ps_u{m}{nh}") for nh in range(2)]
                        for m in range(2)]
                for c in range(2):
                    for m in range(2):
                        for nh in range(2):
                            mm(ps_u[m][nh][:],
                               wp_sb[c][:, 128 * m:128 * (m + 1)],
                               tDst[c][:, 512 * nh:512 * (nh + 1)],
                               start=(c == 0), stop=(c == 1))
                uall = []
                for m in range(2):
                    u = wpool.tile([128, 1024], F16, tag=f"u{m}", name=f"u{m}")
                    for nh in range(2):
                        copy_out(u[:, 512 * nh:512 * (nh + 1)], ps_u[m][nh][:])
                    uall.append(u)
                # q' = sum_k E_k U_k
                ps_q = [ppmm.tile([128, 256], F32, tag="mm", name=f"ps_q{m}")
                        for m in range(2)]
                first = True
                for k in range(4):
                    for c in range(2):
                        for m in range(2):
                            mm(ps_q[m][:],
                               tEsT[k][c][:, 128 * m:128 * (m + 1)],
                               uall[c][:, 256 * k:256 * (k + 1)],
                               start=first, stop=(k == 3 and c == 1))
                        first = False
                q_new = []
                for m in range(2):
                    t = qpool.tile([128, 256], F16, tag=f"q{m}", name=f"q{m}")
                    copy_out(t[:], ps_q[m][:])
                    q_new.append(t)
                q_sb = q_new
                # build next layer's kron tiles while the PE crunches this one
                if l > 0:
                    build_ab(l - 1)

            # ---------------- final contraction ----------------
            h_sb = []
            for m in range(2):
                ps = ppsm.tile([128, B_PER], F32, tag="sm", name="ps_g")
                for c in range(2):
                    nc.tensor.matmul(
                        ps[:], q_sb[c][:, 128 * m:128 * (m + 1)],
                        Pr[c][:], start=(c == 0), stop=(c == 1))
                h = cpool.tile([128, B_PER], F16, tag=f"h{m}", name=f"h{m}")
                nc.vector.tensor_mul(h[:], ps[:], Pc[m][:])
                h_sb.append(h)
            ps_o = ppsm.tile([B_PER, 1], F32, tag="sm", name="ps_o")
            for m in range(2):
                nc.tensor.matmul(ps_o[:], h_sb[m][:], tones[:],
                                 start=(m == 0), stop=(m == 1))
            out_sb = cpool.tile([B_PER, 1], F32, tag="osb", name="osb")
            nc.vector.tensor_copy(out_sb[:], ps_o[:])
            nc.sync.dma_start(out_d[:, :], out_sb[:])

    nc.compile()
    return nc


# ---------------------------------------------------------------------------
# Host entry point
# ---------------------------------------------------------------------------

_NC = None


def _get_nc():
    global _NC
    if _NC is None:
        _NC = build_nc()
    return _NC


def kernel(x: np.ndarray, weights: np.ndarray) -> np.ndarray:
    from concourse.bass_utils import run_bass_kernel_spmd

    nc = _get_nc()
    x = np.ascontiguousarray(x, dtype=np.float32)
    weights = np.ascontiguousarray(weights, dtype=np.float32)
    in_maps = [
        {"xp": x[i * B_PER:(i + 1) * B_PER], "wt": weights}
        for i in range(N_CORES)
    ]
    res = run_bass_kernel_spmd(nc, in_maps, list(range(N_CORES)))
    out = np.concatenate([res.results[i]["out"] for i in range(N_CORES)], axis=0)
    return out.astype(np.float32)


# revision 20
# speedup vs baseline: 1.4377x; 1.0614x over previous
"""Trainium2 Bass kernel for ConfigurableNoisyQuantumLayer.

Math: the circuit is a fixed sequence of single-qubit rotations, CNOTs and
noise channels acting on an 8-qubit density matrix, batched over 32 inputs x.
In the (real) Pauli-transfer-matrix picture every channel is a real 4^n x 4^n
matrix. We pull the observable Z_0 back through the 6 layers (Heisenberg
picture) -- one shared real (4^4)x(4^4)=256x256 matrix chain independent of
the batch -- then contract with per-sample product-state Pauli vectors.

Per adjoint layer (l = 5..0), with q the 256x256 pullback matrix
(rows = wires 0-3 pair-index, cols = wires 4-7):
    T  = A_l q          A_l = kron_{w=0..3} F(l,w)^T
    W  = T B_l          B_l = kron_{w=4..7} F(l,w)
    q' = sum_k E_k W D_k       (k = 0..3: rank-4 Schmidt split of the one
                                row/col-crossing CNOT pair; E_k, D_k are
                                constants that also absorb the row-local and
                                col-local CNOT+noise blocks)
F(l,w) = N1 @ blkdiag(1, Rz(t_z) Ry(t_y)) is the per-wire rotation+noise PTM.
Output: out[b] = P_r(b)^T q P_c(b) with P_r/P_c kron products of per-wire
encoding vectors N1 @ (1, sin x, 0, cos x).

All matmul operands are fp16 (PE runs 1 cycle/row vs 2 for fp32; FWL weight
loads enabled). Constants ship as two blobbed fp16 inline tensors (2 DMAs).
A_l/B_l kron tiles for layer l-1 are built (DVE) while the PE crunches layer
l, so the chain starts as soon as layer 5's tiles exist.

Each of the 8 cores runs the identical chain and handles 4 of the 32 samples.
"""

import os
import sys

import numpy as np

sys.path.insert(0, "/opt/trn_rl_repo")

import concourse.bass as bass  # noqa: E402
import concourse.bacc as bacc  # noqa: E402
import concourse.tile as tile  # noqa: E402
from concourse import mybir  # noqa: E402

F32 = mybir.dt.float32
F16 = mybir.dt.float16
AF = mybir.ActivationFunctionType

N_QUBITS = 8
DEPTH = 6
BATCH = 32
N_CORES = 8
B_PER = BATCH // N_CORES  # 4
G1, G2 = 0.0003, 0.0065

HALF_PI = float(np.pi / 2)

# ---------------------------------------------------------------------------
# Constant precompute (numpy, float64 -> float16)
# ---------------------------------------------------------------------------


def _consts():
    I2 = np.eye(2, dtype=complex)
    X = np.array([[0, 1], [1, 0]], dtype=complex)
    Y = np.array([[0, -1j], [1j, 0]], dtype=complex)
    Z = np.diag([1.0, -1.0]).astype(complex)
    PAULI = [I2, X, Y, Z]

    def amp_k(g):
        return [np.array([[1, 0], [0, np.sqrt(1 - g)]], complex),
                np.array([[0, np.sqrt(g)], [0, 0]], complex)]

    def phase_k(g):
        return [np.array([[1, 0], [0, np.sqrt(1 - g)]], complex),
                np.array([[0, 0], [0, np.sqrt(g)]], complex)]

    def depol_k(p):
        s0, s = np.sqrt(1 - p), np.sqrt(p / 3.0)
        return [s0 * I2, s * X, s * Y, s * Z]

    def super_1q(kraus):
        S = np.zeros((4, 4))
        for a in range(4):
            for b in range(4):
                acc = 0j
                for K in kraus:
                    acc += np.trace(PAULI[a] @ K @ PAULI[b] @ K.conj().T)
                S[a, b] = (0.5 * acc).real
        return S

    def chan(chs):
        S = np.eye(4)
        for k in chs:
            S = super_1q(k) @ S
        return S

    N1 = chan([amp_k(G1 * 0.3), phase_k(G1 * 0.2), depol_k(G1 * 0.5)])
    N2 = chan([amp_k(G2 * 0.3), phase_k(G2 * 0.2), depol_k(G2 * 0.5)])

    CNOT = np.array(
        [[1, 0, 0, 0], [0, 1, 0, 0], [0, 0, 0, 1], [0, 0, 1, 0]], complex)
    S_CNOT = np.zeros((16, 16))
    for a1 in range(4):
        for a2 in range(4):
            PA = np.kron(PAULI[a1], PAULI[a2])
            for b1 in range(4):
                for b2 in range(4):
                    PB = np.kron(PAULI[b1], PAULI[b2])
                    S_CNOT[4 * a1 + a2, 4 * b1 + b2] = (
                        0.25 * np.trace(PA @ CNOT @ PB @ CNOT.conj().T)).real
    C2 = np.kron(N2, N2) @ S_CNOT
    C2T = C2.T

    def lift(M, pos):  # on 4 base-4 digits, digit 0 most significant
        return np.kron(np.kron(np.eye(4 ** pos), M), np.eye(4 ** (2 - pos)))

    G_c = lift(C2T, 0) @ lift(C2T, 1) @ lift(C2T, 2)
    G_r = lift(C2T, 0) @ lift(C2T, 1) @ lift(C2T, 2)  # same structure

    C4 = C2T.reshape(4, 4, 4, 4)
    R = C4.transpose(0, 2, 1, 3).reshape(16, 16)
    U, s, Vt = np.linalg.svd(R)
    rank = int((s > 1e-12).sum())
    assert rank == 4, rank
    Ds = np.zeros((4, 256, 256))
    EsT = np.zeros((4, 256, 256))
    for k in range(4):
        alpha = (np.sqrt(s[k]) * U[:, k]).reshape(4, 4)
        beta = (np.sqrt(s[k]) * Vt[k, :]).reshape(4, 4)
        E_k = G_r @ np.kron(np.eye(64), alpha)
        D_k = (np.kron(beta, np.eye(64)) @ G_c).T
        Ds[k] = D_k
        EsT[k] = E_k.T

    # selectors with N1 fused, padded to K=128 with data rows at partition 32*r
    # (Activation/engine APs must start at 32-aligned partitions)
    M1f = np.zeros((128, 16))
    M2f = np.zeros((128, 16))
    for p in range(16):
        for r in range(4):
            M1f[32 * r, p] = N1[p >> 2, r]
            M2f[32 * r, p] = N1[p & 3, r]
    # encoding: rows (1, sin, cos) -> N1 cols (0, 1, 3)
    M1e = np.zeros((128, 16))
    M2e = np.zeros((128, 16))
    for p in range(16):
        for j, r in enumerate([0, 1, 3]):
            M1e[32 * j, p] = N1[p >> 2, r]
            M2e[32 * j, p] = N1[p & 3, r]
    S16h = np.zeros((2, 16, 128))
    S16t = np.zeros((16, 128))
    for c in range(2):
        for p in range(128):
            S16h[c, (128 * c + p) >> 4, p] = 1.0
    for p in range(128):
        S16t[p & 15, p] = 1.0

    f = np.float16
    # big blob [128, 4160]: fused selectors first (small leading DMA piece),
    # then Dst tiles, EsT tiles
    Dst = np.ascontiguousarray(Ds.transpose(1, 0, 2).reshape(256, 1024))
    big = np.concatenate(
        [M1f, M2f, M1e, M2e]
        + [Dst[0:128], Dst[128:256]]
        + [EsT[k, 128 * c:128 * (c + 1), :] for k in range(4) for c in range(2)],
        axis=1)
    assert big.shape == (128, 4160)
    # selector blob [16, 384]
    sel = np.zeros((16, 384))
    sel[:, 0:128] = S16h[0]
    sel[:, 128:256] = S16h[1]
    sel[:, 256:384] = S16t
    return dict(big=big.astype(f), sel=sel.astype(f))

# ---------------------------------------------------------------------------
# Bass kernel builder
# ---------------------------------------------------------------------------


def build_nc():
    """One NeuronCore program: inputs xp [4,8], wt [6,8,2] -> out [4,1]."""
    C = _consts()

    nc = bacc.Bacc("TRN2", target_bir_lowering=False, debug=False,
                   num_devices=N_CORES)
    xp = nc.declare_dram_parameter("xp", [B_PER, N_QUBITS], F32, isOutput=False)
    wt = nc.declare_dram_parameter("wt", [DEPTH, N_QUBITS, 2], F32, isOutput=False)
    out_d = nc.declare_dram_parameter("out", [B_PER, 1], F32, isOutput=True)

    dBig = nc.inline_tensor(C["big"], "cBig")
    dSel = nc.inline_tensor(C["sel"], "cSel")

    with tile.TileContext(nc) as tc:
        with (
            tc.tile_pool(name="cpool", bufs=1) as cpool,
            tc.tile_pool(name="abpool", bufs=1) as abpool,
            tc.tile_pool(name="wpool", bufs=2) as wpool,
            tc.tile_pool(name="qpool", bufs=2) as qpool,
            tc.tile_pool(name="ppmm", bufs=6, space="PSUM") as ppmm,
            tc.tile_pool(name="ppsm", bufs=2, space="PSUM") as ppsm,
        ):
            # ---- input DMAs first (short; unblock the trig path asap) ----
            th = cpool.tile([1, 96], F32, tag="th", name="th")
            nc.sync.dma_start(
                th[:].rearrange("p (t j) -> p t j", t=2),
                wt[:].rearrange("l w t -> () t (l w)"))
            sx = cpool.tile([1, 32], F32, tag="sx", name="sx")
            nc.sync.dma_start(sx[:], xp[:].rearrange("b w -> () (b w)"))

            # ---- constant blobs (sel + 2-piece big: selectors land early) ----
            selt = cpool.tile([16, 384], F16, tag="sel", name="sel")
            nc.sync.dma_start(selt[:], dSel[:, :])
            bigt = cpool.tile([128, 4160], F16, tag="big", name="big")
            nc.sync.dma_start(bigt[:, 0:64], dBig[:, 0:64])
            nc.sync.dma_start(bigt[:, 64:4160], dBig[:, 64:4160])

            tM1f = bigt[:, 0:16]
            tM2f = bigt[:, 16:32]
            tM1e = bigt[:, 32:48]
            tM2e = bigt[:, 48:64]
            tDst = [bigt[:, 64 + 1024 * c:64 + 1024 * (c + 1)] for c in range(2)]
            tEsT = [[bigt[:, 2112 + 256 * (2 * k + c):2112 + 256 * (2 * k + c + 1)]
                     for c in range(2)] for k in range(4)]
            tS16h = [selt[0:16, 128 * c:128 * (c + 1)] for c in range(2)]
            tS16t = selt[0:16, 256:384]

            tones = cpool.tile([128, 1], F16, tag="ones", name="ones")
            nc.vector.memset(tones[:], 1.0)
            tpi2 = cpool.tile([1, 1], F32, tag="pi2", name="pi2")
            nc.vector.memset(tpi2[:], HALF_PI)

            # ---------------- angles -> rotation-block rows ----------------
            sn = cpool.tile([1, 96], F32, tag="sn", name="sn")
            cs = cpool.tile([1, 96], F32, tag="cs", name="cs")
            nc.scalar.activation(sn[:], th[:], AF.Sin)
            nc.scalar.activation(cs[:], th[:], AF.Sin, bias=tpi2[:])
            sy, szr = sn[0:1, 0:48], sn[0:1, 48:96]
            cy, czr = cs[0:1, 0:48], cs[0:1, 48:96]
            pcc = cpool.tile([1, 48], F16, tag="pcc", name="pcc")  # cz*cy
            pcs = cpool.tile([1, 48], F16, tag="pcs", name="pcs")  # cz*sy
            psc = cpool.tile([1, 48], F16, tag="psc", name="psc")  # sz*cy
            pss = cpool.tile([1, 48], F16, tag="pss", name="pss")  # sz*sy
            nc.vector.tensor_mul(pcc[:], czr, cy)
            nc.vector.tensor_mul(pcs[:], czr, sy)
            nc.vector.tensor_mul(psc[:], szr, cy)
            nc.vector.tensor_mul(pss[:], szr, sy)

            # RVall partitions 32*r hold the rotation-block row r for all (l,w)
            RVall = cpool.tile([128, 192], F16, tag="rvall", name="RVall")
            nc.vector.memset(RVall[:], 0.0)
            rvv = [RVall[32 * r:32 * r + 1, :].rearrange("p (j n) -> p j n", n=4)
                   for r in range(4)]

            def c3(a):
                return a.rearrange("p j -> p j ()")

            nc.vector.memset(rvv[0][:, :, 0:1], 1.0)
            nc.vector.tensor_copy(rvv[1][:, :, 1:2], c3(pcc[:]))
            nc.scalar.mul(rvv[1][:, :, 2:3], c3(szr), -1.0)
            nc.vector.tensor_copy(rvv[1][:, :, 3:4], c3(pcs[:]))
            nc.vector.tensor_copy(rvv[2][:, :, 1:2], c3(psc[:]))
            nc.vector.tensor_copy(rvv[2][:, :, 2:3], c3(czr))
            nc.vector.tensor_copy(rvv[2][:, :, 3:4], c3(pss[:]))
            nc.scalar.mul(rvv[3][:, :, 1:2], c3(sy), -1.0)
            nc.vector.tensor_copy(rvv[3][:, :, 3:4], c3(cy))

            # t1a[p, 4j+n] = F_j[p>>2, n]; t2a[p, 4j+n] = F_j[p&3, n]
            ps1 = ppsm.tile([16, 192], F32, tag="sm", name="ps1")
            nc.tensor.matmul(ps1[:], tM1f, RVall[:], start=True, stop=True)
            t1a = cpool.tile([16, 192], F16, tag="t1a", name="t1a")
            nc.scalar.copy(t1a[:], ps1[:])
            ps2 = ppsm.tile([16, 192], F32, tag="sm", name="ps2")
            nc.tensor.matmul(ps2[:], tM2f, RVall[:], start=True, stop=True)
            t2a = cpool.tile([16, 192], F16, tag="t2a", name="t2a")
            nc.scalar.copy(t2a[:], ps2[:])

            # pair-kron tiles, one batched mul per pos: fpa[pos][p, 16l + 4a+b]
            fpa = []
            for pos in range(4):
                fp = abpool.tile([16, 96], F16, tag=f"fpa{pos}", name=f"fpa{pos}")
                t1v = t1a[:].rearrange("p (l x) -> p l x", l=DEPTH)
                t2v = t2a[:].rearrange("p (l x) -> p l x", l=DEPTH)
                nc.vector.tensor_mul(
                    fp[:].rearrange("p (l a b) -> p l a b", l=DEPTH, a=4),
                    t1v[:, :, 8 * pos:8 * pos + 4].unsqueeze(3)
                       .broadcast_to([16, DEPTH, 4, 4]),
                    t2v[:, :, 8 * pos + 4:8 * pos + 8].unsqueeze(2)
                       .broadcast_to([16, DEPTH, 4, 4]),
                )
                fpa.append(fp)

            # quad selector expansions, batched over layers: zAB[c] = [zA|zB],
            # yAB = [yA|yB]  (A and B side by side so one kron mul covers both)
            zAB = [cpool.tile([128, 192], F16, tag=f"zAB{c}", name=f"zAB{c}")
                   for c in range(2)]
            yAB = cpool.tile([128, 192], F16, tag="yAB", name="yAB")

            def sel_expand(sel_ap, fp_all, dst_ap, tag):
                ps = ppsm.tile([128, 96], F32, tag="sm", name=f"ps{tag}")
                nc.tensor.matmul(ps[:], sel_ap, fp_all[:], start=True, stop=True)
                nc.scalar.copy(dst_ap, ps[:])

            for c in range(2):
                sel_expand(tS16h[c], fpa[0], zAB[c][:, 0:96], f"zA{c}")
                sel_expand(tS16h[c], fpa[2], zAB[c][:, 96:192], f"zB{c}")
            sel_expand(tS16t, fpa[1], yAB[:, 0:96], "yA")
            sel_expand(tS16t, fpa[3], yAB[:, 96:192], "yB")

            # A/B kron tiles for one layer: one [128, 512] mul per (l, c)
            # AB[l][c] = [At-block | Bt-block]
            At = {}
            Bt = {}

            def build_ab(l, c):
                ab = abpool.tile([128, 512], F16, tag=f"AB{l}_{c}",
                                 name=f"AB{l}_{c}")
                zv = zAB[c][:].rearrange("p (s la) -> p s la", s=2)
                yv = yAB[:].rearrange("p (s la) -> p s la", s=2)
                nc.vector.tensor_mul(
                    ab[:].rearrange("p (s a b) -> p s a b", s=2, a=16),
                    zv[:, :, 16 * l:16 * (l + 1)].unsqueeze(3)
                      .broadcast_to([128, 2, 16, 16]),
                    yv[:, :, 16 * l:16 * (l + 1)].unsqueeze(2)
                      .broadcast_to([128, 2, 16, 16]),
                )
                At.setdefault(l, [None, None])[c] = ab[:, 0:256]
                Bt.setdefault(l, [None, None])[c] = ab[:, 256:512]

            build_ab(DEPTH - 1, 0)
            build_ab(DEPTH - 1, 1)

            # ---------------- q init (rank-1: single 1.0 at [192, 0]) --------
            q_sb = []
            for c in range(2):
                t = qpool.tile([128, 256], F16, tag=f"q{c}", name=f"q{c}")
                nc.vector.memset(t[:], 0.0)
                q_sb.append(t)
            nc.vector.memset(q_sb[1][64:65, 0:1], 1.0)

            # ---------------- the 6-layer chain ----------------
            def mm(dst_psum, lhsT, rhs, start, stop):
                nc.tensor.matmul(dst_psum, lhsT, rhs, start=start, stop=stop)

            def vcopy(dst, src):
                nc.vector.tensor_copy(dst, src)

            def scopy(dst, src):
                nc.scalar.copy(dst, src)

            def emit_encoding():
                # encoding vectors -> per-sample Pr/Pc (issued mid-chain; the
                # tiny PE matmuls slot between chain stages)
                enc3 = cpool.tile([128, 32], F16, tag="enc3", name="enc3")
                nc.vector.memset(enc3[:], 0.0)
                nc.vector.memset(enc3[0:1, :], 1.0)
                nc.scalar.activation(enc3[32:33, :], sx[:], AF.Sin)
                nc.scalar.activation(enc3[64:65, :], sx[:], AF.Sin,
                                     bias=tpi2[:])

                pse1 = ppsm.tile([16, 32], F32, tag="sm", name="pse1")
                nc.tensor.matmul(pse1[:], tM1e, enc3[:], start=True, stop=True)
                s1e = cpool.tile([16, 32], F16, tag="s1e", name="s1e")
                nc.scalar.copy(s1e[:], pse1[:])
                pse2 = ppsm.tile([16, 32], F32, tag="sm", name="pse2")
                nc.tensor.matmul(pse2[:], tM2e, enc3[:], start=True, stop=True)
                s2e = cpool.tile([16, 32], F16, tag="s2e", name="s2e")
                nc.scalar.copy(s2e[:], pse2[:])

                def wcol(t, w):
                    return t[:].rearrange("p (b w) -> p b w", w=8)[:, :, w]

                # ahi = [a01 | a45], alo = [a23 | a67]  (cols = 4 samples)
                ahi = cpool.tile([16, 8], F16, tag="ahi", name="ahi")
                alo = cpool.tile([16, 8], F16, tag="alo", name="alo")
                nc.vector.tensor_mul(ahi[:, 0:4], wcol(s1e, 0), wcol(s2e, 1))
                nc.vector.tensor_mul(ahi[:, 4:8], wcol(s1e, 4), wcol(s2e, 5))
                nc.vector.tensor_mul(alo[:, 0:4], wcol(s1e, 2), wcol(s2e, 3))
                nc.vector.tensor_mul(alo[:, 4:8], wcol(s1e, 6), wcol(s2e, 7))

                psy = ppsm.tile([128, 8], F32, tag="sm", name="psy")
                nc.tensor.matmul(psy[:], tS16t, alo[:], start=True, stop=True)
                yq = cpool.tile([128, 8], F16, tag="yq", name="yq")
                nc.scalar.copy(yq[:], psy[:])
                Pr = []
                Pc = []
                for c in range(2):
                    psz = ppsm.tile([128, 8], F32, tag="sm", name="psz")
                    nc.tensor.matmul(psz[:], tS16h[c], ahi[:],
                                     start=True, stop=True)
                    pr = cpool.tile([128, B_PER], F16, tag=f"pr{c}",
                                    name=f"pr{c}")
                    nc.vector.tensor_mul(pr[:], psz[:, 0:4], yq[:, 0:4])
                    pc = cpool.tile([128, B_PER], F16, tag=f"pc{c}",
                                    name=f"pc{c}")
                    nc.vector.tensor_mul(pc[:], psz[:, 4:8], yq[:, 4:8])
                    Pr.append(pr)
                    Pc.append(pc)
                return Pr, Pc

            Pr = Pc = None
            for s in range(DEPTH):
                l = DEPTH - 1 - s
                # Tp = q^T @ A  (m-outer: tile m's accumulation closes early so
                # its copy overlaps the remaining matmuls)
                ps_tp = [ppmm.tile([128, 256], F32, tag="mm", name=f"ps_tp{m}")
                         for m in range(2)]
                tp_sb = [wpool.tile([128, 256], F16, tag=f"tp{m}", name=f"tp{m}")
                         for m in range(2)]
                for m in range(2):
                    for c in range(2):
                        mm(ps_tp[m][:], q_sb[c][:, 128 * m:128 * (m + 1)],
                           At[l][c], start=(c == 0), stop=(c == 1))
                    (scopy if m == 0 else vcopy)(tp_sb[m][:], ps_tp[m][:])
                if l > 0:
                    build_ab(l - 1, 0)
                # Wp = B^T @ Tp
                ps_wp = [ppmm.tile([128, 256], F32, tag="mm", name=f"ps_wp{m}")
                         for m in range(2)]
                wp_sb = [wpool.tile([128, 256], F16, tag=f"wp{m}", name=f"wp{m}")
                         for m in range(2)]
                for m in range(2):
                    for c in range(2):
                        mm(ps_wp[m][:], Bt[l][c][:, 128 * m:128 * (m + 1)],
                           tp_sb[c][:], start=(c == 0), stop=(c == 1))
                    (scopy if m == 0 else vcopy)(wp_sb[m][:], ps_wp[m][:])
                if l > 0:
                    build_ab(l - 1, 1)
                # U = W @ [D_0|D_1|D_2|D_3]
                ps_u = [[ppmm.tile([128, 512], F32, tag="mm",
                                   name=f"ps_u{m}{nh}") for nh in range(2)]
                        for m in range(2)]
                uall = [wpool.tile([128, 1024], F16, tag=f"u{m}", name=f"u{m}")
                        for m in range(2)]
                for m in range(2):
                    for nh in range(2):
                        for c in range(2):
                            mm(ps_u[m][nh][:],
                               wp_sb[c][:, 128 * m:128 * (m + 1)],
                               tDst[c][:, 512 * nh:512 * (nh + 1)],
                               start=(c == 0), stop=(c == 1))
                        (scopy if nh == 0 else vcopy)(
                            uall[m][:, 512 * nh:512 * (nh + 1)],
                            ps_u[m][nh][:])
                # q' = sum_k E_k U_k
                ps_q = [ppmm.tile([128, 256], F32, tag="mm", name=f"ps_q{m}")
                        for m in range(2)]
                q_new = [qpool.tile([128, 256], F16, tag=f"q{m}", name=f"q{m}")
                         for m in range(2)]
                for m in range(2):
                    for k in range(4):
                        for c in range(2):
                            mm(ps_q[m][:],
                               tEsT[k][c][:, 128 * m:128 * (m + 1)],
                               uall[c][:, 256 * k:256 * (k + 1)],
                               start=(k == 0 and c == 0),
                               stop=(k == 3 and c == 1))
                    (scopy if m == 0 else vcopy)(q_new[m][:], ps_q[m][:])
                q_sb = q_new
                if s == 0:
                    Pr, Pc = emit_encoding()

            # ---------------- final contraction ----------------
            h_sb = []
            for m in range(2):
                ps = ppsm.tile([128, B_PER], F32, tag="sm", name="ps_g")
                for c in range(2):
                    nc.tensor.matmul(
                        ps[:], q_sb[c][:, 128 * m:128 * (m + 1)],
                        Pr[c][:], start=(c == 0), stop=(c == 1))
                h = cpool.tile([128, B_PER], F16, tag=f"h{m}", name=f"h{m}")
                nc.vector.tensor_mul(h[:], ps[:], Pc[m][:])
                h_sb.append(h)
            ps_o = ppsm.tile([B_PER, 1], F32, tag="sm", name="ps_o")
            for m in range(2):
                nc.tensor.matmul(ps_o[:], h_sb[m][:], tones[:],
                                 start=(m == 0), stop=(m == 1))
            out_sb = cpool.tile([B_PER, 1], F32, tag="osb", name="osb")
            nc.vector.tensor_copy(out_sb[:], ps_o[:])
            nc.sync.dma_start(out_d[:, :], out_sb[:])

    nc.compile()
    return nc


# ---------------------------------------------------------------------------
# Host entry point
# ---------------------------------------------------------------------------

_NC = None


def _get_nc():
    global _NC
    if _NC is None:
        _NC = build_nc()
    return _NC


def kernel(x: np.ndarray, weights: np.ndarray) -> np.ndarray:
    from concourse.bass_utils import run_bass_kernel_spmd

    nc = _get_nc()
    x = np.ascontiguousarray(x, dtype=np.float32)
    weights = np.ascontiguousarray(weights, dtype=np.float32)
    in_maps = [
        {"xp": x[i * B_PER:(i + 1) * B_PER], "wt": weights}
        for i in range(N_CORES)
    ]
    res = run_bass_kernel_spmd(nc, in_maps, list(range(N_CORES)))
    out = np.concatenate([res.results[i]["out"] for i in range(N_CORES)], axis=0)
    return out.astype(np.float32)


# revision 26
# speedup vs baseline: 1.4544x; 1.0116x over previous
"""Trainium2 Bass kernel for ConfigurableNoisyQuantumLayer.

Math: the circuit is a fixed sequence of single-qubit rotations, CNOTs and
noise channels acting on an 8-qubit density matrix, batched over 32 inputs x.
In the (real) Pauli-transfer-matrix picture every channel is a real 4^n x 4^n
matrix. We pull the observable Z_0 back through the 6 layers (Heisenberg
picture) -- one shared real (4^4)x(4^4)=256x256 matrix chain independent of
the batch -- then contract with per-sample product-state Pauli vectors.

Per adjoint layer (l = 5..0), with q the 256x256 pullback matrix
(rows = wires 0-3 pair-index, cols = wires 4-7):
    T  = A_l q          A_l = kron_{w=0..3} F(l,w)^T
    W  = T B_l          B_l = kron_{w=4..7} F(l,w)
    q' = sum_k E_k W D_k       (k = 0..3: rank-4 Schmidt split of the one
                                row/col-crossing CNOT pair; E_k, D_k are
                                constants that also absorb the row-local and
                                col-local CNOT+noise blocks)
F(l,w) = N1 @ blkdiag(1, Rz(t_z) Ry(t_y)) is the per-wire rotation+noise PTM.
Output: out[b] = P_r(b)^T q P_c(b) with P_r/P_c kron products of per-wire
encoding vectors N1 @ (1, sin x, 0, cos x).

All matmul operands are fp16 (PE runs 1 cycle/row vs 2 for fp32; FWL weight
loads enabled). Constants ship as two blobbed fp16 inline tensors (2 DMAs).
A_l/B_l kron tiles for layer l-1 are built (DVE) while the PE crunches layer
l, so the chain starts as soon as layer 5's tiles exist.

Each of the 8 cores runs the identical chain and handles 4 of the 32 samples.
"""

import os
import sys

import numpy as np

sys.path.insert(0, "/opt/trn_rl_repo")

import concourse.bass as bass  # noqa: E402
import concourse.bacc as bacc  # noqa: E402
import concourse.tile as tile  # noqa: E402
from concourse import mybir  # noqa: E402

F32 = mybir.dt.float32
F16 = mybir.dt.float16
AF = mybir.ActivationFunctionType

N_QUBITS = 8
DEPTH = 6
BATCH = 32
N_CORES = 8
B_PER = BATCH // N_CORES  # 4
G1, G2 = 0.0003, 0.0065

HALF_PI = float(np.pi / 2)

# ---------------------------------------------------------------------------
# Constant precompute (numpy, float64 -> float16)
# ---------------------------------------------------------------------------


def _consts():
    I2 = np.eye(2, dtype=complex)
    X = np.array([[0, 1], [1, 0]], dtype=complex)
    Y = np.array([[0, -1j], [1j, 0]], dtype=complex)
    Z = np.diag([1.0, -1.0]).astype(complex)
    PAULI = [I2, X, Y, Z]

    def amp_k(g):
        return [np.array([[1, 0], [0, np.sqrt(1 - g)]], complex),
                np.array([[0, np.sqrt(g)], [0, 0]], complex)]

    def phase_k(g):
        return [np.array([[1, 0], [0, np.sqrt(1 - g)]], complex),
                np.array([[0, 0], [0, np.sqrt(g)]], complex)]

    def depol_k(p):
        s0, s = np.sqrt(1 - p), np.sqrt(p / 3.0)
        return [s0 * I2, s * X, s * Y, s * Z]

    def super_1q(kraus):
        S = np.zeros((4, 4))
        for a in range(4):
            for b in range(4):
                acc = 0j
                for K in kraus:
                    acc += np.trace(PAULI[a] @ K @ PAULI[b] @ K.conj().T)
                S[a, b] = (0.5 * acc).real
        return S

    def chan(chs):
        S = np.eye(4)
        for k in chs:
            S = super_1q(k) @ S
        return S

    N1 = chan([amp_k(G1 * 0.3), phase_k(G1 * 0.2), depol_k(G1 * 0.5)])
    N2 = chan([amp_k(G2 * 0.3), phase_k(G2 * 0.2), depol_k(G2 * 0.5)])

    CNOT = np.array(
        [[1, 0, 0, 0], [0, 1, 0, 0], [0, 0, 0, 1], [0, 0, 1, 0]], complex)
    S_CNOT = np.zeros((16, 16))
    for a1 in range(4):
        for a2 in range(4):
            PA = np.kron(PAULI[a1], PAULI[a2])
            for b1 in range(4):
                for b2 in range(4):
                    PB = np.kron(PAULI[b1], PAULI[b2])
                    S_CNOT[4 * a1 + a2, 4 * b1 + b2] = (
                        0.25 * np.trace(PA @ CNOT @ PB @ CNOT.conj().T)).real
    C2 = np.kron(N2, N2) @ S_CNOT
    C2T = C2.T

    def lift(M, pos):  # on 4 base-4 digits, digit 0 most significant
        return np.kron(np.kron(np.eye(4 ** pos), M), np.eye(4 ** (2 - pos)))

    G_c = lift(C2T, 0) @ lift(C2T, 1) @ lift(C2T, 2)
    G_r = lift(C2T, 0) @ lift(C2T, 1) @ lift(C2T, 2)  # same structure

    C4 = C2T.reshape(4, 4, 4, 4)
    R = C4.transpose(0, 2, 1, 3).reshape(16, 16)
    U, s, Vt = np.linalg.svd(R)
    rank = int((s > 1e-12).sum())
    assert rank == 4, rank
    Ds = np.zeros((4, 256, 256))
    EsT = np.zeros((4, 256, 256))
    for k in range(4):
        alpha = (np.sqrt(s[k]) * U[:, k]).reshape(4, 4)
        beta = (np.sqrt(s[k]) * Vt[k, :]).reshape(4, 4)
        E_k = G_r @ np.kron(np.eye(64), alpha)
        D_k = (np.kron(beta, np.eye(64)) @ G_c).T
        Ds[k] = D_k
        EsT[k] = E_k.T

    # selectors with N1 fused, padded to K=128 with data rows at partition 32*r
    # (Activation/engine APs must start at 32-aligned partitions)
    M1f = np.zeros((128, 16))
    M2f = np.zeros((128, 16))
    for p in range(16):
        for r in range(4):
            M1f[32 * r, p] = N1[p >> 2, r]
            M2f[32 * r, p] = N1[p & 3, r]
    # encoding: rows (1, sin, cos) -> N1 cols (0, 1, 3)
    M1e = np.zeros((128, 16))
    M2e = np.zeros((128, 16))
    for p in range(16):
        for j, r in enumerate([0, 1, 3]):
            M1e[32 * j, p] = N1[p >> 2, r]
            M2e[32 * j, p] = N1[p & 3, r]
    S16h = np.zeros((2, 16, 128))
    S16t = np.zeros((16, 128))
    for c in range(2):
        for p in range(128):
            S16h[c, (128 * c + p) >> 4, p] = 1.0
    for p in range(128):
        S16t[p & 15, p] = 1.0

    f = np.float16
    # big blob [128, 4160]: fused selectors first (small leading DMA piece),
    # then Dst tiles, EsT tiles
    Dst = np.ascontiguousarray(Ds.transpose(1, 0, 2).reshape(256, 1024))
    big = np.concatenate(
        [M1f, M2f, M1e, M2e]
        + [Dst[0:128], Dst[128:256]]
        + [EsT[k, 128 * c:128 * (c + 1), :] for k in range(4) for c in range(2)],
        axis=1)
    assert big.shape == (128, 4160)
    # selector blob [16, 384]
    sel = np.zeros((16, 384))
    sel[:, 0:128] = S16h[0]
    sel[:, 128:256] = S16h[1]
    sel[:, 256:384] = S16t
    return dict(big=big.astype(f), sel=sel.astype(f))

# ---------------------------------------------------------------------------
# Bass kernel builder
# ---------------------------------------------------------------------------


def build_nc():
    """One NeuronCore program: inputs xp [4,8], wt [6,8,2] -> out [4,1]."""
    C = _consts()

    nc = bacc.Bacc("TRN2", target_bir_lowering=False, debug=False,
                   num_devices=N_CORES)
    xp = nc.declare_dram_parameter("xp", [B_PER, N_QUBITS], F32, isOutput=False)
    wt = nc.declare_dram_parameter("wt", [DEPTH, N_QUBITS, 2], F32, isOutput=False)
    out_d = nc.declare_dram_parameter("out", [B_PER, 1], F32, isOutput=True)

    dBig = nc.inline_tensor(C["big"], "cBig")
    dSel = nc.inline_tensor(C["sel"], "cSel")

    with tile.TileContext(nc) as tc:
        with (
            tc.tile_pool(name="cpool", bufs=1) as cpool,
            tc.tile_pool(name="abpool", bufs=1) as abpool,
            tc.tile_pool(name="wpool", bufs=2) as wpool,
            tc.tile_pool(name="qpool", bufs=2) as qpool,
            tc.tile_pool(name="ppmm", bufs=6, space="PSUM") as ppmm,
            tc.tile_pool(name="ppsm", bufs=2, space="PSUM") as ppsm,
        ):
            # ---- input DMAs first (short; unblock the trig path asap) ----
            th = cpool.tile([1, 96], F32, tag="th", name="th")
            nc.sync.dma_start(
                th[:].rearrange("p (t j) -> p t j", t=2),
                wt[:].rearrange("l w t -> () t (l w)"))
            sx = cpool.tile([1, 32], F32, tag="sx", name="sx")
            nc.sync.dma_start(sx[:], xp[:].rearrange("b w -> () (b w)"))

            # ---- constant blobs (sel + 2-piece big: selectors land early) ----
            selt = cpool.tile([16, 384], F16, tag="sel", name="sel")
            nc.sync.dma_start(selt[:], dSel[:, :])
            bigt = cpool.tile([128, 4160], F16, tag="big", name="big")
            nc.sync.dma_start(bigt[:, 0:64], dBig[:, 0:64])
            nc.sync.dma_start(bigt[:, 64:4160], dBig[:, 64:4160])

            tM1f = bigt[:, 0:16]
            tM2f = bigt[:, 16:32]
            tM1e = bigt[:, 32:48]
            tM2e = bigt[:, 48:64]
            tDst = [bigt[:, 64 + 1024 * c:64 + 1024 * (c + 1)] for c in range(2)]
            tEsT = [[bigt[:, 2112 + 256 * (2 * k + c):2112 + 256 * (2 * k + c + 1)]
                     for c in range(2)] for k in range(4)]
            tS16h = [selt[0:16, 128 * c:128 * (c + 1)] for c in range(2)]
            tS16t = selt[0:16, 256:384]

            tones = cpool.tile([128, 1], F16, tag="ones", name="ones")
            nc.vector.memset(tones[:], 1.0)
            tpi2 = cpool.tile([1, 1], F32, tag="pi2", name="pi2")
            nc.vector.memset(tpi2[:], HALF_PI)

            # PE warm-up: dummy matmuls on (uninitialized) scratch keep the
            # HAM activity monitor busy while the DMA/trig preamble runs, so
            # the chain starts at 2.4 GHz instead of 1.2 GHz.
            scr = cpool.tile([128, 256], F16, tag="scr", name="scr")
            nc.vector.memset(scr[:], 0.5)
            ps_warm = ppmm.tile([128, 512], F32, tag="mm", name="ps_warm")
            for _ in range(12):
                nc.tensor.matmul(ps_warm[:, 0:256], scr[:, 0:128], scr[:],
                                 start=True, stop=True)

            # ---------------- angles -> rotation-block rows ----------------
            sn = cpool.tile([1, 96], F32, tag="sn", name="sn")
            cs = cpool.tile([1, 96], F32, tag="cs", name="cs")
            nc.scalar.activation(sn[:], th[:], AF.Sin)
            nc.scalar.activation(cs[:], th[:], AF.Sin, bias=tpi2[:])
            sy, szr = sn[0:1, 0:48], sn[0:1, 48:96]
            cy, czr = cs[0:1, 0:48], cs[0:1, 48:96]
            pcc = cpool.tile([1, 48], F16, tag="pcc", name="pcc")  # cz*cy
            pcs = cpool.tile([1, 48], F16, tag="pcs", name="pcs")  # cz*sy
            psc = cpool.tile([1, 48], F16, tag="psc", name="psc")  # sz*cy
            pss = cpool.tile([1, 48], F16, tag="pss", name="pss")  # sz*sy
            nc.vector.tensor_mul(pcc[:], czr, cy)
            nc.vector.tensor_mul(pcs[:], czr, sy)
            nc.vector.tensor_mul(psc[:], szr, cy)
            nc.vector.tensor_mul(pss[:], szr, sy)

            # RVall partitions 32*r hold the rotation-block row r for all (l,w)
            RVall = cpool.tile([128, 192], F16, tag="rvall", name="RVall")
            nc.vector.memset(RVall[:], 0.0)
            rvv = [RVall[32 * r:32 * r + 1, :].rearrange("p (j n) -> p j n", n=4)
                   for r in range(4)]

            def c3(a):
                return a.rearrange("p j -> p j ()")

            nc.vector.memset(rvv[0][:, :, 0:1], 1.0)
            nc.vector.tensor_copy(rvv[1][:, :, 1:2], c3(pcc[:]))
            nc.scalar.mul(rvv[1][:, :, 2:3], c3(szr), -1.0)
            nc.vector.tensor_copy(rvv[1][:, :, 3:4], c3(pcs[:]))
            nc.vector.tensor_copy(rvv[2][:, :, 1:2], c3(psc[:]))
            nc.vector.tensor_copy(rvv[2][:, :, 2:3], c3(czr))
            nc.vector.tensor_copy(rvv[2][:, :, 3:4], c3(pss[:]))
            nc.scalar.mul(rvv[3][:, :, 1:2], c3(sy), -1.0)
            nc.vector.tensor_copy(rvv[3][:, :, 3:4], c3(cy))

            # t1a[p, 4j+n] = F_j[p>>2, n]; t2a[p, 4j+n] = F_j[p&3, n]
            ps1 = ppsm.tile([16, 192], F32, tag="sm", name="ps1")
            nc.tensor.matmul(ps1[:], tM1f, RVall[:], start=True, stop=True)
            t1a = cpool.tile([16, 192], F16, tag="t1a", name="t1a")
            nc.scalar.copy(t1a[:], ps1[:])
            ps2 = ppsm.tile([16, 192], F32, tag="sm", name="ps2")
            nc.tensor.matmul(ps2[:], tM2f, RVall[:], start=True, stop=True)
            t2a = cpool.tile([16, 192], F16, tag="t2a", name="t2a")
            nc.vector.tensor_copy(t2a[:], ps2[:])

            # pair-kron tiles, one batched mul per pos: fpa[pos][p, 16l + 4a+b]
            fpa = []
            for pos in range(4):
                fp = abpool.tile([16, 96], F16, tag=f"fpa{pos}", name=f"fpa{pos}")
                t1v = t1a[:].rearrange("p (l x) -> p l x", l=DEPTH)
                t2v = t2a[:].rearrange("p (l x) -> p l x", l=DEPTH)
                nc.vector.tensor_mul(
                    fp[:].rearrange("p (l a b) -> p l a b", l=DEPTH, a=4),
                    t1v[:, :, 8 * pos:8 * pos + 4].unsqueeze(3)
                       .broadcast_to([16, DEPTH, 4, 4]),
                    t2v[:, :, 8 * pos + 4:8 * pos + 8].unsqueeze(2)
                       .broadcast_to([16, DEPTH, 4, 4]),
                )
                fpa.append(fp)

            # quad selector expansions, batched over layers: zAB[c] = [zA|zB],
            # yAB = [yA|yB]  (A and B side by side so one kron mul covers both)
            zAB = [cpool.tile([128, 192], F16, tag=f"zAB{c}", name=f"zAB{c}")
                   for c in range(2)]
            yAB = cpool.tile([128, 192], F16, tag="yAB", name="yAB")

            sel_flip = [0]

            def sel_expand(sel_ap, fp_all, dst_ap, tag):
                ps = ppsm.tile([128, 96], F32, tag="sm", name=f"ps{tag}")
                nc.tensor.matmul(ps[:], sel_ap, fp_all[:], start=True, stop=True)
                if sel_flip[0] % 2 == 0:
                    nc.scalar.copy(dst_ap, ps[:])
                else:
                    nc.vector.tensor_copy(dst_ap, ps[:])
                sel_flip[0] += 1

            # yAB first: both AB-mul operands want yAB; zAB[0] right after
            sel_expand(tS16t, fpa[1], yAB[:, 0:96], "yA")
            sel_expand(tS16h[0], fpa[0], zAB[0][:, 0:96], "zA0")
            sel_expand(tS16t, fpa[3], yAB[:, 96:192], "yB")
            sel_expand(tS16h[0], fpa[2], zAB[0][:, 96:192], "zB0")
            sel_expand(tS16h[1], fpa[0], zAB[1][:, 0:96], "zA1")
            sel_expand(tS16h[1], fpa[2], zAB[1][:, 96:192], "zB1")

            # A/B kron tiles for one layer: one [128, 512] mul per (l, c)
            # AB[l][c] = [At-block | Bt-block]
            At = {}
            Bt = {}

            def build_ab(l, c):
                ab = abpool.tile([128, 512], F16, tag=f"AB{l}_{c}",
                                 name=f"AB{l}_{c}")
                zv = zAB[c][:].rearrange("p (s la) -> p s la", s=2)
                yv = yAB[:].rearrange("p (s la) -> p s la", s=2)
                nc.vector.tensor_mul(
                    ab[:].rearrange("p (s a b) -> p s a b", s=2, a=16),
                    zv[:, :, 16 * l:16 * (l + 1)].unsqueeze(3)
                      .broadcast_to([128, 2, 16, 16]),
                    yv[:, :, 16 * l:16 * (l + 1)].unsqueeze(2)
                      .broadcast_to([128, 2, 16, 16]),
                )
                At.setdefault(l, [None, None])[c] = ab[:, 0:256]
                Bt.setdefault(l, [None, None])[c] = ab[:, 256:512]

            build_ab(DEPTH - 1, 0)
            build_ab(DEPTH - 1, 1)

            # ---------------- q init (rank-1: single 1.0 at [192, 0]) --------
            q_sb = []
            for c in range(2):
                t = qpool.tile([128, 256], F16, tag=f"q{c}", name=f"q{c}")
                nc.vector.memset(t[:], 0.0)
                q_sb.append(t)
            nc.vector.memset(q_sb[1][64:65, 0:1], 1.0)

            # ---------------- the 6-layer chain ----------------
            def mm(dst_psum, lhsT, rhs, start, stop):
                nc.tensor.matmul(dst_psum, lhsT, rhs, start=start, stop=stop)

            def vcopy(dst, src):
                nc.vector.tensor_copy(dst, src)

            def scopy(dst, src):
                nc.scalar.copy(dst, src)

            def emit_encoding():
                # encoding vectors -> per-sample Pr/Pc (issued mid-chain; the
                # tiny PE matmuls slot between chain stages)
                enc3 = cpool.tile([128, 32], F16, tag="enc3", name="enc3")
                nc.vector.memset(enc3[:], 0.0)
                nc.vector.memset(enc3[0:1, :], 1.0)
                nc.scalar.activation(enc3[32:33, :], sx[:], AF.Sin)
                nc.scalar.activation(enc3[64:65, :], sx[:], AF.Sin,
                                     bias=tpi2[:])

                pse1 = ppsm.tile([16, 32], F32, tag="sm", name="pse1")
                nc.tensor.matmul(pse1[:], tM1e, enc3[:], start=True, stop=True)
                s1e = cpool.tile([16, 32], F16, tag="s1e", name="s1e")
                nc.scalar.copy(s1e[:], pse1[:])
                pse2 = ppsm.tile([16, 32], F32, tag="sm", name="pse2")
                nc.tensor.matmul(pse2[:], tM2e, enc3[:], start=True, stop=True)
                s2e = cpool.tile([16, 32], F16, tag="s2e", name="s2e")
                nc.scalar.copy(s2e[:], pse2[:])

                def wcol(t, w):
                    return t[:].rearrange("p (b w) -> p b w", w=8)[:, :, w]

                # ahi = [a01 | a45], alo = [a23 | a67]  (cols = 4 samples)
                ahi = cpool.tile([16, 8], F16, tag="ahi", name="ahi")
                alo = cpool.tile([16, 8], F16, tag="alo", name="alo")
                nc.vector.tensor_mul(ahi[:, 0:4], wcol(s1e, 0), wcol(s2e, 1))
                nc.vector.tensor_mul(ahi[:, 4:8], wcol(s1e, 4), wcol(s2e, 5))
                nc.vector.tensor_mul(alo[:, 0:4], wcol(s1e, 2), wcol(s2e, 3))
                nc.vector.tensor_mul(alo[:, 4:8], wcol(s1e, 6), wcol(s2e, 7))

                psy = ppsm.tile([128, 8], F32, tag="sm", name="psy")
                nc.tensor.matmul(psy[:], tS16t, alo[:], start=True, stop=True)
                yq = cpool.tile([128, 8], F16, tag="yq", name="yq")
                nc.scalar.copy(yq[:], psy[:])
                Pr = []
                Pc = []
                for c in range(2):
                    psz = ppsm.tile([128, 8], F32, tag="sm", name="psz")
                    nc.tensor.matmul(psz[:], tS16h[c], ahi[:],
                                     start=True, stop=True)
                    pr = cpool.tile([128, B_PER], F16, tag=f"pr{c}",
                                    name=f"pr{c}")
                    nc.vector.tensor_mul(pr[:], psz[:, 0:4], yq[:, 0:4])
                    pc = cpool.tile([128, B_PER], F16, tag=f"pc{c}",
                                    name=f"pc{c}")
                    nc.vector.tensor_mul(pc[:], psz[:, 4:8], yq[:, 4:8])
                    Pr.append(pr)
                    Pc.append(pc)
                return Pr, Pc

            Pr = Pc = None
            for s in range(DEPTH):
                l = DEPTH - 1 - s
                # Tp = q^T @ A  (m-outer: tile m's accumulation closes early so
                # its copy overlaps the remaining matmuls)
                ps_tp = [ppmm.tile([128, 256], F32, tag="mm", name=f"ps_tp{m}")
                         for m in range(2)]
                tp_sb = [wpool.tile([128, 256], F16, tag=f"tp{m}", name=f"tp{m}")
                         for m in range(2)]
                for m in range(2):
                    for c in range(2):
                        mm(ps_tp[m][:], q_sb[c][:, 128 * m:128 * (m + 1)],
                           At[l][c], start=(c == 0), stop=(c == 1))
                    # tp0 on the faster engine (Wp's first matmuls need it)
                    (vcopy if m == 0 else scopy)(tp_sb[m][:], ps_tp[m][:])
                if l > 0:
                    build_ab(l - 1, 0)
                # Wp = B^T @ Tp
                ps_wp = [ppmm.tile([128, 256], F32, tag="mm", name=f"ps_wp{m}")
                         for m in range(2)]
                wp_sb = [wpool.tile([128, 256], F16, tag=f"wp{m}", name=f"wp{m}")
                         for m in range(2)]
                for m in range(2):
                    for c in range(2):
                        mm(ps_wp[m][:], Bt[l][c][:, 128 * m:128 * (m + 1)],
                           tp_sb[c][:], start=(c == 0), stop=(c == 1))
                    # U consumes wp[c] in [128,128] column quarters; copy the
                    # critical low half on V, the late-needed high half on S
                    vcopy(wp_sb[m][:, 0:128], ps_wp[m][:, 0:128])
                    scopy(wp_sb[m][:, 128:256], ps_wp[m][:, 128:256])
                if l > 0:
                    build_ab(l - 1, 1)
                # U = W @ [D_0|D_1|D_2|D_3]
                ps_u = [[ppmm.tile([128, 512], F32, tag="mm",
                                   name=f"ps_u{m}{nh}") for nh in range(2)]
                        for m in range(2)]
                uall = [wpool.tile([128, 1024], F16, tag=f"u{m}", name=f"u{m}")
                        for m in range(2)]
                for m in range(2):
                    for nh in range(2):
                        for c in range(2):
                            mm(ps_u[m][nh][:],
                               wp_sb[c][:, 128 * m:128 * (m + 1)],
                               tDst[c][:, 512 * nh:512 * (nh + 1)],
                               start=(c == 0), stop=(c == 1))
                        if m == 0:
                            # u0 halves finish early; full-width copies hide
                            (scopy if nh == 0 else vcopy)(
                                uall[m][:, 512 * nh:512 * (nh + 1)],
                                ps_u[m][nh][:])
                        else:
                            # u1 is needed k-block-by-k-block right as q'
                            # starts: quarter copies alternating S/V
                            o = 512 * nh
                            scopy(uall[m][:, o:o + 256],
                                  ps_u[m][nh][:, 0:256])
                            vcopy(uall[m][:, o + 256:o + 512],
                                  ps_u[m][nh][:, 256:512])
                # q' = sum_k E_k U_k
                ps_q = [ppmm.tile([128, 256], F32, tag="mm", name=f"ps_q{m}")
                        for m in range(2)]
                q_new = [qpool.tile([128, 256], F16, tag=f"q{m}", name=f"q{m}")
                         for m in range(2)]
                for m in range(2):
                    for k in range(4):
                        for c in range(2):
                            mm(ps_q[m][:],
                               tEsT[k][c][:, 128 * m:128 * (m + 1)],
                               uall[c][:, 256 * k:256 * (k + 1)],
                               start=(k == 0 and c == 0),
                               stop=(k == 3 and c == 1))
                    if m == 0:
                        scopy(q_new[m][:], ps_q[m][:])
                    else:
                        # q1 closes last; halve its latency across V+S
                        vcopy(q_new[m][:, 0:128], ps_q[m][:, 0:128])
                        scopy(q_new[m][:, 128:256], ps_q[m][:, 128:256])
                q_sb = q_new
                if s == 0:
                    Pr, Pc = emit_encoding()

            # ---------------- final contraction ----------------
            h_sb = []
            for m in range(2):
                ps = ppsm.tile([128, B_PER], F32, tag="sm", name="ps_g")
                for c in range(2):
                    nc.tensor.matmul(
                        ps[:], q_sb[c][:, 128 * m:128 * (m + 1)],
                        Pr[c][:], start=(c == 0), stop=(c == 1))
                h = cpool.tile([128, B_PER], F16, tag=f"h{m}", name=f"h{m}")
                nc.vector.tensor_mul(h[:], ps[:], Pc[m][:])
                h_sb.append(h)
            ps_o = ppsm.tile([B_PER, 1], F32, tag="sm", name="ps_o")
            for m in range(2):
                nc.tensor.matmul(ps_o[:], h_sb[m][:], tones[:],
                                 start=(m == 0), stop=(m == 1))
            out_sb = cpool.tile([B_PER, 1], F32, tag="osb", name="osb")
            nc.vector.tensor_copy(out_sb[:], ps_o[:])
            nc.sync.dma_start(out_d[:, :], out_sb[:])

    nc.compile()
    return nc


# ---------------------------------------------------------------------------
# Host entry point
# ---------------------------------------------------------------------------

_NC = None


def _get_nc():
    global _NC
    if _NC is None:
        _NC = build_nc()
    return _NC


def kernel(x: np.ndarray, weights: np.ndarray) -> np.ndarray:
    from concourse.bass_utils import run_bass_kernel_spmd

    nc = _get_nc()
    x = np.ascontiguousarray(x, dtype=np.float32)
    weights = np.ascontiguousarray(weights, dtype=np.float32)
    in_maps = [
        {"xp": x[i * B_PER:(i + 1) * B_PER], "wt": weights}
        for i in range(N_CORES)
    ]
    res = run_bass_kernel_spmd(nc, in_maps, list(range(N_CORES)))
    out = np.concatenate([res.results[i]["out"] for i in range(N_CORES)], axis=0)
    return out.astype(np.float32)
